# revision 4
# baseline (speedup 1.0000x reference)
"""Self-contained Trainium2 Bass kernel for nn_GCNMagnetModel
(3-layer GCN: N=50000 nodes, E=600000 edges, H=128, 64 graphs,
8 NeuronCores, SPMD single NEFF).

Sharding: nodes/edges sharded by graph id (graphs 8k..8k+7 -> core k), so
segment pools are core-local; weight matrices replicated. Uniform
blocks-per-graph layout keeps the SPMD program shape identical on every
core; all per-core variation lives in the input data (indices, masks).

Host does index/layout work only (sorting, partitioning, padding, local
renumbering, masks, integer degree histogram of the edge list); all FP
math on tensor values runs on device:
 - deg -> rsqrt on device; per-layer bf16 tables t1 = (h@W)*dinv[src].
 - per layer: table shard AllGather (HBM, Shared output), then per
   dst-block message passing: dma_gather of edge source rows (src-sorted,
   split across 4 SWDGE queues for DMA parallelism - the gathers are the
   dominant cost), one-hot matmuls accumulate agg[feature, dst] on the PE
   (gathered rows as lhsT, DVE-built one-hot as rhs), self-loop via one
   identity matmul per block, epilogue = TT * dinv[dst] + ACT tanh with
   per-feature bias.
 - masked max/mean pooling via per-graph reduces, small head matmul.

kernel(**inputs) -> [64, 41] float32.
"""
import numpy as np
import ml_dtypes
from contextlib import ExitStack

import concourse.tile as tile
import concourse.mybir as mybir
from concourse import bacc
from concourse import library_config
from concourse.bass_utils import run_bass_kernel_spmd

NCORE = 8
P = 128
GPC = 8


def wrap16(v):  # [n] -> [128, n/16]: idx[i%16, i//16] tiled 8x
    a = v.reshape(-1, 16).T
    return np.tile(a, (8, 1)).copy()


def prep(x, edge_index, batch, n_graphs=64, sort_src=True):
    N = x.shape[0]
    batch = np.asarray(batch)
    src_g, dst_g = np.asarray(edge_index[0]), np.asarray(edge_index[1])
    E = src_g.shape[0]

    gstart = np.searchsorted(batch, np.arange(n_graphs), side="left")
    gend = np.searchsorted(batch, np.arange(n_graphs), side="right")
    gsz = gend - gstart

    gblk_u = max(int((gsz.max() + P - 1) // P), 1)
    NBLK = GPC * gblk_u
    NMAXP = NBLK * P
    HALF = 4 * NMAXP
    assert HALF < 32768, f"HALF={HALF} exceeds int16"

    node_core = batch // GPC
    node_loc = (batch % GPC) * (gblk_u * P) + (np.arange(N) - gstart[batch])
    node_row = node_core * NMAXP + (node_loc % P) * NBLK + node_loc // P

    # degrees (host integer histogram; +1 for self loop)
    deg = np.bincount(dst_g, minlength=N).astype(np.float32) + 1.0

    # edges -> owner core by dst
    e_core = node_core[dst_g]
    e_dstloc = node_loc[dst_g]
    e_blk = e_dstloc // P
    e_dl = e_dstloc % P
    e_row = node_row[src_g]
    e_half = (e_row >= HALF).astype(np.int64)

    cnts = np.zeros((NCORE, NBLK, 2), np.int64)
    np.add.at(cnts, (e_core, e_blk, e_half), 1)
    cAm = ((cnts[:, :, 0].max(axis=0) + P - 1) // P).astype(np.int64)  # [NBLK]
    cBm = ((cnts[:, :, 1].max(axis=0) + P - 1) // P).astype(np.int64)
    aoff = np.r_[0, np.cumsum(cAm)]  # [NBLK+1] chunk offsets in A region
    boff = np.r_[0, np.cumsum(cBm)]
    CAT, CBT = int(aoff[-1]), int(boff[-1])
    NCHT = CAT + CBT
    cT = cAm + cBm
    off_bm = np.r_[0, np.cumsum(cT)]  # block-major chunk offsets

    # fill per-core idx / dstloc
    # sort by src row within (core, blk, half): monotonic DMA addresses
    order = (np.lexsort((e_row, e_half, e_blk, e_core)) if sort_src
             else np.lexsort((e_half, e_blk, e_core)))
    so_core, so_blk, so_half = e_core[order], e_blk[order], e_half[order]
    so_row, so_dl = e_row[order], e_dl[order]
    key = (so_core * NBLK + so_blk) * 2 + so_half
    runstart = np.r_[0, np.flatnonzero(np.diff(key)) + 1]
    runid = np.zeros(E, np.int64)
    runid[runstart[1:]] = 1
    runid = np.cumsum(runid)
    pos_in_run = np.arange(E) - runstart[runid]

    slotA = aoff[so_blk] * P + pos_in_run  # valid where so_half==0
    slotB = boff[so_blk] * P + pos_in_run
    slot = np.where(so_half == 0, slotA, CAT * P + slotB)
    idxv = np.where(so_half == 0, so_row, so_row - HALF).astype(np.int16)

    idx_all = np.zeros((NCORE, NCHT * P), np.int16)
    dl_all = np.full((NCORE, NCHT * P), -1.0, np.float32)
    idx_all[so_core, slot] = idxv
    dl_all[so_core, slot] = so_dl

    # per-graph valid mask (same layout every core)
    cores = []
    for k in range(NCORE):
        sel = node_core == k
        loc_k = node_loc[sel]
        xk = np.zeros((NMAXP, 2), np.float32)
        xk[loc_k] = np.asarray(x)[sel]
        degk = np.ones(NMAXP, np.float32)
        degk[loc_k] = deg[sel]
        degT = np.ones((P, NBLK), np.float32)
        degT[loc_k % P, loc_k // P] = deg[sel]
        padmask = np.zeros(NMAXP, np.float32)
        padmask[loc_k] = 1.0

        # dstloc block-major [128, NCHT]
        dA = dl_all[k, : CAT * P].reshape(CAT, P)
        dB = dl_all[k, CAT * P :].reshape(CBT, P)
        cols = []
        for b in range(NBLK):
            if cAm[b]:
                cols.append(dA[aoff[b] : aoff[b + 1]])
            if cBm[b]:
                cols.append(dB[boff[b] : boff[b + 1]])
        dstloc_bm = (
            np.concatenate(cols, 0).T if cols else np.zeros((P, 0), np.float32)
        )  # [128, NCHT]

        gv = (gsz[k * GPC : (k + 1) * GPC] > 0).astype(np.float32)
        cores.append(
            dict(
                xT=np.ascontiguousarray(xk.T).astype(ml_dtypes.bfloat16),
                degT=degT,
                degxT=np.tile(degk[None, :], (2, 1)).astype(np.float32),
                idx=wrap16(idx_all[k]),
                dstloc=np.ascontiguousarray(dstloc_bm).astype(ml_dtypes.bfloat16),
                padmask=np.tile(padmask[None, :], (P, 1)).astype(ml_dtypes.bfloat16),
                gvalid=np.tile(gv[None, :], (P, 1)).astype(np.float32),
            )
        )

    return dict(
        NBLK=NBLK,
        NMAXP=NMAXP,
        HALF=HALF,
        gblk_u=gblk_u,
        cAm=cAm,
        cBm=cBm,
        aoff=aoff,
        boff=boff,
        CAT=CAT,
        CBT=CBT,
        NCHT=NCHT,
        cT=cT,
        off_bm=off_bm,
        gsz=gsz,
        cores=cores,
        node_core=node_core,
        node_loc=node_loc,
        node_row=node_row,
        deg=deg,
    )


F32 = mybir.dt.float32
BF16 = mybir.dt.bfloat16
I16 = mybir.dt.int16
AF = mybir.ActivationFunctionType
OP = mybir.AluOpType

NCORE = 8
P = 128
GPC = 8
H = 128
OC = 41


def build(meta, GBLK=7, SINGLE_PACKET=False, ABLATE=(), REPS=1, HOST_OH=False, QUEUES=1, GBUFS=2, QSPLIT=1):
    NBLK, NMAXP, HALF = meta["NBLK"], meta["NMAXP"], meta["HALF"]
    cAm, cBm = [int(v) for v in meta["cAm"]], [int(v) for v in meta["cBm"]]
    aoff, boff = [int(v) for v in meta["aoff"]], [int(v) for v in meta["boff"]]
    CAT, CBT, NCHT = meta["CAT"], meta["CBT"], meta["NCHT"]
    cT = [int(v) for v in meta["cT"]]
    off_bm = [int(v) for v in meta["off_bm"]]
    gblk_u = meta["gblk_u"]
    NTAB = NCORE * NMAXP
    ngrp = (NBLK + GBLK - 1) // GBLK

    nc = bacc.Bacc(None, target_bir_lowering=False, num_swdge_queues=QUEUES)

    # ---- IO ----
    xT_d = nc.dram_tensor("xT", [2, NMAXP], BF16, kind="ExternalInput")
    degT_d = nc.dram_tensor("degT", [128, NBLK], BF16, kind="ExternalInput")
    degxT_d = nc.dram_tensor("degxT", [2, NMAXP], BF16, kind="ExternalInput")
    idx_d = nc.dram_tensor("idx", [128, NCHT * 8], I16, kind="ExternalInput")
    dstloc_d = nc.dram_tensor("dstloc", [128, NCHT], BF16, kind="ExternalInput")
    colidx_d = nc.dram_tensor("colidx", [128, 128], BF16, kind="ExternalInput")
    identbf_d = nc.dram_tensor("identbf", [128, 128], BF16, kind="ExternalInput")
    padmask_d = nc.dram_tensor("padmask", [128, NMAXP], BF16, kind="ExternalInput")
    gvalid_d = nc.dram_tensor("gvalid", [128, GPC], F32, kind="ExternalInput")
    gcnt_d = nc.dram_tensor("gcnt", [128, GPC], F32, kind="ExternalInput")
    W1_d = nc.dram_tensor("W1", [2, H], F32, kind="ExternalInput")
    W2_d = nc.dram_tensor("W2", [H, H], F32, kind="ExternalInput")
    W3_d = nc.dram_tensor("W3", [H, H], F32, kind="ExternalInput")
    Wo_d = nc.dram_tensor("Wo", [H, 2, OC], F32, kind="ExternalInput")
    bvec_d = nc.dram_tensor("bvec", [128, 3], F32, kind="ExternalInput")
    bo_d = nc.dram_tensor("bo", [GPC, OC], F32, kind="ExternalInput")
    ohost_d = (nc.dram_tensor("ohost", [128, NCHT * 128], BF16,
                              kind="ExternalInput") if HOST_OH else None)
    out_d = nc.dram_tensor("out", [GPC, OC], F32, kind="ExternalOutput")

    shard_d = [nc.dram_tensor(f"shard{L}", [NMAXP, H], BF16) for L in range(3)]
    table_d = [
        nc.dram_tensor(f"table{L}", [NTAB, H], BF16, addr_space="Shared")
        for L in range(3)
    ]

    PB = 4  # prep batch (blocks per PSUM tile)

    with tile.TileContext(nc) as tc, ExitStack() as ctx:
        const = ctx.enter_context(tc.tile_pool(name="const", bufs=1))
        resid = ctx.enter_context(tc.tile_pool(name="resid", bufs=1))
        gap = ctx.enter_context(tc.tile_pool(name="gap", bufs=GBUFS))
        gbp = ctx.enter_context(tc.tile_pool(name="gbp", bufs=GBUFS))
        ohp = ctx.enter_context(tc.tile_pool(name="ohp", bufs=3))
        wk = ctx.enter_context(tc.tile_pool(name="wk", bufs=3))
        poolbig = ctx.enter_context(tc.tile_pool(name="poolbig", bufs=1))
        prepps = ctx.enter_context(tc.tile_pool(name="prepps", bufs=2, space="PSUM"))
        aggps = ctx.enter_context(tc.tile_pool(name="aggps", bufs=4, space="PSUM"))
        headps = ctx.enter_context(tc.tile_pool(name="headps", bufs=1, space="PSUM"))

        nc.gpsimd.load_library(library_config.mlp)

        def load_const(dram, shape, dt):
            t = const.tile(shape, dt, tag=dram.name)
            nc.sync.dma_start(t[:], dram[:])
            return t

        xT_t = load_const(xT_d, [2, NMAXP], BF16)
        degT_t = load_const(degT_d, [128, NBLK], BF16)
        degxT_t = load_const(degxT_d, [2, NMAXP], BF16)
        idx_t = load_const(idx_d, [128, NCHT * 8], I16)
        dstloc_t = load_const(dstloc_d, [128, NCHT], BF16)
        colidx_t = load_const(colidx_d, [128, 128], BF16)
        identbf_t = load_const(identbf_d, [128, 128], BF16)
        padmask_t = load_const(padmask_d, [128, NMAXP], BF16)
        gvalid_t = load_const(gvalid_d, [128, GPC], F32)
        gcnt_t = load_const(gcnt_d, [128, GPC], F32)
        W1_t = load_const(W1_d, [2, H], F32)
        W2_t = load_const(W2_d, [H, H], F32)
        W3_t = load_const(W3_d, [H, H], F32)
        Wo_t = load_const(Wo_d, [H, 2, OC], F32)
        bvec_t = load_const(bvec_d, [128, 3], F32)
        bo_t = load_const(bo_d, [GPC, OC], F32)

        # ---- P0: dinv, y0, bf16 weights ----
        rec1 = wk.tile([128, NBLK], F32, tag="rec1")
        nc.vector.reciprocal(rec1[:], degT_t[:])
        dinv_t = resid.tile([128, NBLK], F32, tag="dinv")
        nc.scalar.sqrt(dinv_t[:], rec1[:])

        rec2 = poolbig.tile([2, NMAXP], BF16, tag="big")
        with nc.allow_low_precision(reason="1/deg of small exact ints; 0.4% ok"):
            nc.vector.reciprocal(rec2[:], degxT_t[:])
        dinvxT = poolbig.tile([2, NMAXP], BF16, tag="big2")
        nc.scalar.sqrt(dinvxT[:], rec2[:])

        y0 = poolbig.tile([2, NMAXP], BF16, tag="big")
        nc.vector.tensor_tensor(y0[:], xT_t[:], dinvxT[:], OP.mult)

        dinv_rep = resid.tile([128, NMAXP], BF16, tag="dinv_rep")
        nc.gpsimd.partition_broadcast(dinv_rep[:], dinvxT[0:1, :])

        W1b = const.tile([2, H], BF16, tag="W1b")
        nc.vector.tensor_copy(W1b[:], W1_t[:])
        W2b = const.tile([128, H], BF16, tag="W2b")
        nc.vector.tensor_copy(W2b[:], W2_t[:])
        W3b = const.tile([128, H], BF16, tag="W3b")
        nc.vector.tensor_copy(W3b[:], W3_t[:])

        sbuild = resid.tile([128, NBLK, H], BF16, tag="sbuild")
        y_t = resid.tile([128, NMAXP], BF16, tag="y")

        # ---- layers (REPS>1 repeats the whole body for marginal timing) ----
        for _rep in range(REPS):
         for L in range(3):
             Wb = (W1b, W2b, W3b)[L]
             # prep: sbuild[:, b, :] = t1 (node-major, bf16)
             for b0 in range(0, NBLK, PB):
                 b1 = min(b0 + PB, NBLK)
                 nb = b1 - b0
                 pp = prepps.tile([128, PB, H], F32, tag="pp")
                 for b in range(b0, b1):
                     lhs = y0[:, b * 128 : (b + 1) * 128] if L == 0 else \
                         y_t[:, b * 128 : (b + 1) * 128]
                     nc.tensor.matmul(pp[:, b - b0, :], lhs, Wb[:],
                                      start=True, stop=True)
                 if L == 0:
                     # dinv already folded into y0 = x * dinv[src]
                     nc.vector.tensor_copy(sbuild[:, b0:b1, :], pp[:, :nb, :])
                 else:
                     nc.vector.tensor_tensor(
                         sbuild[:, b0:b1, :], pp[:, :nb, :],
                         dinv_t[:, b0:b1, None].broadcast_to((128, nb, H)),
                         OP.mult,
                     )
             nc.sync.dma_start(
                 shard_d[L].rearrange("(p b) h -> p (b h)", b=NBLK)[:, :],
                 sbuild[:].rearrange("p b h -> p (b h)"),
             )
             if "ag" not in ABLATE:
                 nc.gpsimd.collective_compute(
                     "AllGather", OP.bypass,
                     replica_groups=[list(range(NCORE))],
                     ins=[shard_d[L][:]], outs=[table_d[L][:]],
                 )

             # message pass
             gAmax = max(aoff[min(g * GBLK + GBLK, NBLK)] - aoff[g * GBLK]
                         for g in range(ngrp))
             gBmax = max(boff[min(g * GBLK + GBLK, NBLK)] - boff[g * GBLK]
                         for g in range(ngrp))
             for g in range(ngrp):
                 b0, b1 = g * GBLK, min((g + 1) * GBLK, NBLK)
                 nA = (aoff[b1] - aoff[b0]) * 128
                 nB = (boff[b1] - boff[b0]) * 128
                 need_g = "gather" not in ABLATE or "mm" not in ABLATE
                 gA = (gap.tile([128, gAmax, H], BF16, tag="gA", name="gA")
                       if gAmax and need_g else None)
                 gB = (gbp.tile([128, gBmax, H], BF16, tag="gB", name="gB")
                       if gBmax and need_g else None)
                 qg = 2 * g * QSPLIT
                 if nA and "gather" not in ABLATE:
                     ncha = nA // 128
                     for s in range(QSPLIT):
                         c0, c1 = (ncha * s) // QSPLIT, (ncha * (s + 1)) // QSPLIT
                         if c1 > c0:
                             nc.gpsimd.dma_gather(
                                 gA[:, c0:c1, :], table_d[L][0:HALF, :],
                                 idx_t[:, (aoff[b0] + c0) * 8 : (aoff[b0] + c1) * 8],
                                 (c1 - c0) * 128, (c1 - c0) * 128, H,
                                 single_packet=SINGLE_PACKET,
                                 queue_num=(qg + s) % QUEUES,
                             )
                 if nB and "gather" not in ABLATE:
                     nchb = nB // 128
                     for s in range(QSPLIT):
                         c0, c1 = (nchb * s) // QSPLIT, (nchb * (s + 1)) // QSPLIT
                         if c1 > c0:
                             nc.gpsimd.dma_gather(
                                 gB[:, c0:c1, :], table_d[L][HALF:, :],
                                 idx_t[:, (CAT + boff[b0] + c0) * 8 : (CAT + boff[b0] + c1) * 8],
                                 (c1 - c0) * 128, (c1 - c0) * 128, H,
                                 single_packet=SINGLE_PACKET,
                                 queue_num=(qg + QSPLIT + s) % QUEUES,
                             )
                 for b in range(b0, b1):
                     nch = cT[b] if "mm" not in ABLATE else 0
                     if nch:
                         oh = ohp.tile([128, max(cT), 128], BF16, tag="oh", name="oh")
                         if HOST_OH:
                             nc.sync.dma_start(
                                 oh[:, :nch, :].rearrange("p c d -> p (c d)"),
                                 ohost_d[:, off_bm[b] * 128 : (off_bm[b] + nch) * 128],
                             )
                         else:
                             nc.vector.tensor_tensor(
                                 oh[:, :nch, :],
                                 colidx_t[:, None, :].broadcast_to((128, nch, 128)),
                                 dstloc_t[:, off_bm[b] : off_bm[b] + nch, None]
                                 .broadcast_to((128, nch, 128)),
                                 OP.is_equal,
                             )
                     ap = aggps.tile([128, 128], F32, tag="agg")
                     for j in range(cAm[b] if "mm" not in ABLATE else 0):
                         nc.tensor.matmul(
                             ap[:], gA[:, aoff[b] - aoff[b0] + j, :],
                             oh[:, j, :], start=(j == 0), stop=False,
                         )
                     for j in range(cBm[b] if "mm" not in ABLATE else 0):
                         nc.tensor.matmul(
                             ap[:], gB[:, boff[b] - boff[b0] + j, :],
                             oh[:, cAm[b] + j, :],
                             start=(cAm[b] == 0 and j == 0), stop=False,
                         )
                     nc.tensor.matmul(
                         ap[:], sbuild[:, b, :], identbf_t[:],
                         start=(nch == 0 or "mm" in ABLATE), stop=True,
                     )
                     z = wk.tile([128, 128], F32, tag="z")
                     nc.vector.tensor_tensor(
                         z[:], ap[:], dinv_rep[:, b * 128 : (b + 1) * 128],
                         OP.mult,
                     )
                     nc.scalar.activation(
                         y_t[:, b * 128 : (b + 1) * 128], z[:], AF.Tanh,
                         bias=bvec_t[:, L : L + 1],
                     )

         # ---- pooling + head ----
         gb = gblk_u * 128
         mx = resid.tile([128, GPC], F32, tag="mx")
         sm = resid.tile([128, GPC], F32, tag="sm")
         for g in range(GPC):
             zg = wk.tile([128, gb], F32, tag="zg")
             nc.vector.tensor_scalar(
                 zg[:], y_t[:, g * gb : (g + 1) * gb], 2.0, None, OP.add)
             nc.vector.tensor_tensor(
                 zg[:], zg[:], padmask_t[:, g * gb : (g + 1) * gb], OP.mult)
             nc.vector.tensor_reduce(
                 mx[:, g : g + 1], zg[:], mybir.AxisListType.X, OP.max)
             nc.vector.tensor_reduce(
                 sm[:, g : g + 1], zg[:], mybir.AxisListType.X, OP.add)
         recg = wk.tile([128, GPC], F32, tag="recg")
         nc.vector.reciprocal(recg[:], gcnt_t[:])
         mean2 = resid.tile([128, GPC], F32, tag="mean2")
         nc.vector.tensor_tensor(mean2[:], sm[:], recg[:], OP.mult)
         nc.vector.tensor_scalar(mean2[:], mean2[:], 2.0, None, OP.subtract)
         nc.vector.tensor_tensor(mean2[:], mean2[:], gvalid_t[:], OP.mult)
         mx2 = resid.tile([128, GPC], F32, tag="mx2")
         nc.vector.tensor_scalar(mx2[:], mx[:], 2.0, None, OP.subtract)
         nc.vector.tensor_tensor(mx2[:], mx2[:], gvalid_t[:], OP.mult)

         headp = headps.tile([GPC, OC], F32, tag="head")
         nc.tensor.matmul(headp[:], mx2[:], Wo_t[:, 0, :], start=True, stop=False)
         nc.tensor.matmul(headp[:], mean2[:], Wo_t[:, 1, :], start=False, stop=True)
         hsum = wk.tile([GPC, OC], F32, tag="hsum")
         nc.vector.tensor_tensor(hsum[:], headp[:], bo_t[:], OP.add)
         ofin = wk.tile([GPC, OC], F32, tag="ofin")
         nc.scalar.activation(ofin[:], hsum[:], AF.Tanh)
         nc.sync.dma_start(out_d[:], ofin[:])

    nc.compile()
    return nc


def make_in_maps(meta, inputs, host_oh=False):
    colidx = np.tile(np.arange(128, dtype=np.float32), (128, 1)).astype(
        ml_dtypes.bfloat16)
    identbf = np.eye(128, dtype=np.float32).astype(ml_dtypes.bfloat16)
    bvec = np.stack(
        [np.asarray(inputs[b], np.float32) for b in ("b1", "b2", "b3")], 1)
    bo_t = np.tile(np.asarray(inputs["bo"], np.float32), (GPC, 1))
    Wo = np.asarray(inputs["Wo"], np.float32)
    Wo_t = np.ascontiguousarray(np.stack([Wo[:H], Wo[H:]], axis=1))
    gsz = meta["gsz"]
    maps = []
    for k, c in enumerate(meta["cores"]):
        gcnt = np.maximum(gsz[k * GPC : (k + 1) * GPC].astype(np.float32), 1.0)
        maps.append({
            "xT": np.asarray(c["xT"]),
            "degT": np.asarray(c["degT"]).astype(ml_dtypes.bfloat16),
            "degxT": np.asarray(c["degxT"]).astype(ml_dtypes.bfloat16),
            "idx": np.asarray(c["idx"]),
            "dstloc": np.asarray(c["dstloc"]),
            "colidx": colidx,
            "identbf": identbf,
            "padmask": np.asarray(c["padmask"]),
            **({"ohost": np.asarray(c["ohost"])} if host_oh else {}),
            "gvalid": np.asarray(c["gvalid"]),
            "gcnt": np.tile(gcnt[None, :], (128, 1)).astype(np.float32),
            "W1": np.asarray(inputs["W1"], np.float32),
            "W2": np.asarray(inputs["W2"], np.float32),
            "W3": np.asarray(inputs["W3"], np.float32),
            "Wo": Wo_t,
            "bvec": bvec.astype(np.float32),
            "bo": bo_t,
        })
    return maps


_CACHE = {}


def kernel(x, edge_index, batch, W1, b1, W2, b2, W3, b3, Wo, bo):
    x = np.asarray(x, np.float32)
    meta = prep(x, np.asarray(edge_index), np.asarray(batch), 64)
    key = (meta["NBLK"], tuple(meta["cAm"]), tuple(meta["cBm"]))
    if key not in _CACHE:
        _CACHE[key] = build(meta, QUEUES=4, QSPLIT=2)
    nc = _CACHE[key]
    inputs = dict(W1=W1, b1=b1, W2=W2, b2=b2, W3=W3, b3=b3, Wo=Wo, bo=bo)
    in_maps = make_in_maps(meta, inputs)
    res = run_bass_kernel_spmd(nc, in_maps, core_ids=list(range(8)), trace=False)
    out = np.concatenate([res.results[k]["out"] for k in range(8)], 0)
    return np.ascontiguousarray(out, dtype=np.float32)



# revision 5
# speedup vs baseline: 1.2526x; 1.2526x over previous
"""Self-contained Trainium2 Bass kernel for nn_GCNMagnetModel
(3-layer GCN: N=50000 nodes, E=600000 edges, H=128, 64 graphs,
8 NeuronCores, SPMD single NEFF).

Sharding: nodes/edges sharded by graph id (graphs 8k..8k+7 -> core k), so
segment pools are core-local; weight matrices replicated. Per layer:
bf16 table t1=(h@W)*dinv[src] -> shard AllGather (HBM Shared) -> per
dst-block dma_gather of edge src rows + one-hot matmuls accumulate
agg[feature,dst] on the PE -> epilogue *dinv[dst] + tanh. Pad-trimmed
gather streams:

Edge streams per (half A/B) use EXACT per-(dst-block) counts padded only to
the max over cores (not rounded to 128). Chunk boundaries no longer align
with dst-block boundaries: a boundary chunk is gathered once but appears in
both adjacent blocks' one-hot matmuls, with the out-of-block lanes masked
to -1 in that block's dstloc columns (is_equal never matches -> zero
column). Cuts gather descriptors ~11-15% at the cost of ~1 extra matmul
per block boundary.
"""
import numpy as np
import ml_dtypes
from contextlib import ExitStack

import concourse.tile as tile
import concourse.mybir as mybir
from concourse import bacc
from concourse import library_config
from concourse.bass_utils import run_bass_kernel_spmd

NCORE = 8
P = 128
GPC = 8
H = 128
OC = 41

F32 = mybir.dt.float32
BF16 = mybir.dt.bfloat16
I16 = mybir.dt.int16
AF = mybir.ActivationFunctionType
OP = mybir.AluOpType


def wrap16(v):  # [n] -> [128, n/16]: idx[i%16, i//16] tiled 8x
    a = v.reshape(-1, 16).T
    return np.tile(a, (8, 1)).copy()


def prep(x, edge_index, batch, n_graphs=64):
    N = x.shape[0]
    batch = np.asarray(batch)
    src_g, dst_g = np.asarray(edge_index[0]), np.asarray(edge_index[1])
    E = src_g.shape[0]

    gstart = np.searchsorted(batch, np.arange(n_graphs), side="left")
    gend = np.searchsorted(batch, np.arange(n_graphs), side="right")
    gsz = gend - gstart

    gblk_u = max(int((gsz.max() + P - 1) // P), 1)
    NBLK = GPC * gblk_u
    NMAXP = NBLK * P
    HALF = 4 * NMAXP
    assert HALF < 32768

    node_core = batch // GPC
    node_loc = (batch % GPC) * (gblk_u * P) + (np.arange(N) - gstart[batch])
    node_row = node_core * NMAXP + (node_loc % P) * NBLK + node_loc // P

    deg = np.bincount(dst_g, minlength=N).astype(np.float32) + 1.0

    e_core = node_core[dst_g]
    e_dstloc = node_loc[dst_g]
    e_blk = e_dstloc // P
    e_dl = e_dstloc % P
    e_row = node_row[src_g]
    e_half = (e_row >= HALF).astype(np.int64)

    cnts = np.zeros((NCORE, NBLK, 2), np.int64)
    np.add.at(cnts, (e_core, e_blk, e_half), 1)
    mA = cnts[:, :, 0].max(axis=0)     # exact max-over-core counts
    mB = cnts[:, :, 1].max(axis=0)
    eoffA = np.r_[0, np.cumsum(mA)]    # edge-unit offsets
    eoffB = np.r_[0, np.cumsum(mB)]
    EA, EB = int(eoffA[-1]), int(eoffB[-1])
    CA, CB = (EA + P - 1) // P, (EB + P - 1) // P
    # chunk ranges per block (may share boundary chunks)
    ca0 = (eoffA[:-1] // P).astype(np.int64)
    ca1 = ((eoffA[1:] + P - 1) // P).astype(np.int64)
    cb0 = (eoffB[:-1] // P).astype(np.int64)
    cb1 = ((eoffB[1:] + P - 1) // P).astype(np.int64)
    nchA = np.where(mA > 0, ca1 - ca0, 0)
    nchB = np.where(mB > 0, cb1 - cb0, 0)
    nch = nchA + nchB
    dOff = np.r_[0, np.cumsum(nch)]    # per-block dstloc column offsets
    DT = int(dOff[-1])

    # per-core edge slot assignment at exact offsets
    order = np.lexsort((e_row, e_half, e_blk, e_core))
    so_core, so_blk, so_half = e_core[order], e_blk[order], e_half[order]
    so_row, so_dl = e_row[order], e_dl[order]
    key = (so_core * NBLK + so_blk) * 2 + so_half
    runstart = np.r_[0, np.flatnonzero(np.diff(key)) + 1]
    runid = np.zeros(E, np.int64)
    runid[runstart[1:]] = 1
    runid = np.cumsum(runid)
    pos_in_run = np.arange(E) - runstart[runid]

    slotA = eoffA[so_blk] + pos_in_run
    slotB = eoffB[so_blk] + pos_in_run
    slot = np.where(so_half == 0, slotA, CA * P + slotB)
    idxv = np.where(so_half == 0, so_row, so_row - HALF).astype(np.int16)

    NCHT = CA + CB
    idx_all = np.zeros((NCORE, NCHT * P), np.int16)
    dl_all = np.full((NCORE, NCHT * P), -1.0, np.float32)
    sl_full = np.where(so_half == 0, slotA, EA + slotB)  # for stream dl
    # idx stream: A region [0, CA*P), B region [CA*P, ...)
    idx_all[so_core, slot] = idxv

    cores = []
    for k in range(NCORE):
        sel = node_core == k
        loc_k = node_loc[sel]
        xk = np.zeros((NMAXP, 2), np.float32)
        xk[loc_k] = np.asarray(x)[sel]
        degk = np.ones(NMAXP, np.float32)
        degk[loc_k] = deg[sel]
        degT = np.ones((P, NBLK), np.float32)
        degT[loc_k % P, loc_k // P] = deg[sel]
        padmask = np.zeros(NMAXP, np.float32)
        padmask[loc_k] = 1.0

        # dl streams (exact positions) for this core
        dlA = np.full(CA * P, -1.0, np.float32)
        dlB = np.full(CB * P, -1.0, np.float32)
        mk = so_core == k
        a = mk & (so_half == 0)
        b = mk & (so_half == 1)
        dlA[slotA[a]] = so_dl[a]
        dlB[slotB[b]] = so_dl[b]

        # per-block dstloc columns with out-of-block lanes masked to -1
        dcols = np.full((DT, P), -1.0, np.float32)
        e_idx = np.arange(P)
        for bb in range(NBLK):
            col = int(dOff[bb])
            if mA[bb] > 0:
                for j in range(int(nchA[bb])):
                    c = int(ca0[bb]) + j
                    seg = dlA[c * P : (c + 1) * P].copy()
                    epos = c * P + e_idx
                    seg[(epos < eoffA[bb]) | (epos >= eoffA[bb + 1])] = -1.0
                    dcols[col + j] = seg
                col += int(nchA[bb])
            if mB[bb] > 0:
                for j in range(int(nchB[bb])):
                    c = int(cb0[bb]) + j
                    seg = dlB[c * P : (c + 1) * P].copy()
                    epos = c * P + e_idx
                    seg[(epos < eoffB[bb]) | (epos >= eoffB[bb + 1])] = -1.0
                    dcols[col + j] = seg

        gv = (gsz[k * GPC : (k + 1) * GPC] > 0).astype(np.float32)
        cores.append(
            dict(
                xT=np.ascontiguousarray(xk.T).astype(ml_dtypes.bfloat16),
                degT=degT,
                degxT=np.tile(degk[None, :], (2, 1)).astype(np.float32),
                idx=wrap16(idx_all[k]),
                dstloc=np.ascontiguousarray(dcols.T).astype(ml_dtypes.bfloat16),
                padmask=np.tile(padmask[None, :], (P, 1)).astype(ml_dtypes.bfloat16),
                gvalid=np.tile(gv[None, :], (P, 1)).astype(np.float32),
            )
        )

    return dict(
        NBLK=NBLK, NMAXP=NMAXP, HALF=HALF, gblk_u=gblk_u,
        mA=mA, mB=mB, eoffA=eoffA, eoffB=eoffB, EA=EA, EB=EB,
        CA=CA, CB=CB, ca0=ca0, ca1=ca1, cb0=cb0, cb1=cb1,
        nchA=nchA, nchB=nchB, nch=nch, dOff=dOff, DT=DT, NCHT=NCHT,
        gsz=gsz, cores=cores, deg=deg,
    )


def build(meta, GBLK=7, SINGLE_PACKET=False, ABLATE=(), REPS=1, QUEUES=4,
          GBUFS=2, QSPLIT=2):
    NBLK, NMAXP, HALF = meta["NBLK"], meta["NMAXP"], meta["HALF"]
    CA, CB, NCHT, DT = meta["CA"], meta["CB"], meta["NCHT"], meta["DT"]
    ca0 = [int(v) for v in meta["ca0"]]
    ca1 = [int(v) for v in meta["ca1"]]
    cb0 = [int(v) for v in meta["cb0"]]
    cb1 = [int(v) for v in meta["cb1"]]
    nchA = [int(v) for v in meta["nchA"]]
    nchB = [int(v) for v in meta["nchB"]]
    dOff = [int(v) for v in meta["dOff"]]
    gblk_u = meta["gblk_u"]
    NTAB = NCORE * NMAXP
    ngrp = (NBLK + GBLK - 1) // GBLK

    nc = bacc.Bacc(None, target_bir_lowering=False, num_swdge_queues=QUEUES)

    xT_d = nc.dram_tensor("xT", [2, NMAXP], BF16, kind="ExternalInput")
    degT_d = nc.dram_tensor("degT", [128, NBLK], BF16, kind="ExternalInput")
    degxT_d = nc.dram_tensor("degxT", [2, NMAXP], BF16, kind="ExternalInput")
    idx_d = nc.dram_tensor("idx", [128, NCHT * 8], I16, kind="ExternalInput")
    dstloc_d = nc.dram_tensor("dstloc", [128, DT], BF16, kind="ExternalInput")
    colidx_d = nc.dram_tensor("colidx", [128, 128], BF16, kind="ExternalInput")
    identbf_d = nc.dram_tensor("identbf", [128, 128], BF16, kind="ExternalInput")
    padmask_d = nc.dram_tensor("padmask", [128, NMAXP], BF16, kind="ExternalInput")
    gvalid_d = nc.dram_tensor("gvalid", [128, GPC], F32, kind="ExternalInput")
    gcnt_d = nc.dram_tensor("gcnt", [128, GPC], F32, kind="ExternalInput")
    W1_d = nc.dram_tensor("W1", [2, H], F32, kind="ExternalInput")
    W2_d = nc.dram_tensor("W2", [H, H], F32, kind="ExternalInput")
    W3_d = nc.dram_tensor("W3", [H, H], F32, kind="ExternalInput")
    Wo_d = nc.dram_tensor("Wo", [H, 2, OC], F32, kind="ExternalInput")
    bvec_d = nc.dram_tensor("bvec", [128, 3], F32, kind="ExternalInput")
    bo_d = nc.dram_tensor("bo", [GPC, OC], F32, kind="ExternalInput")
    out_d = nc.dram_tensor("out", [GPC, OC], F32, kind="ExternalOutput")

    shard_d = [nc.dram_tensor(f"shard{L}", [NMAXP, H], BF16) for L in range(3)]
    table_d = [
        nc.dram_tensor(f"table{L}", [NTAB, H], BF16, addr_space="Shared")
        for L in range(3)
    ]

    PB = 4
    # group gather chunk ranges
    gA0 = [ca0[g * GBLK] for g in range(ngrp)]
    gA1 = [ca1[min(g * GBLK + GBLK, NBLK) - 1] for g in range(ngrp)]
    gB0 = [cb0[g * GBLK] for g in range(ngrp)]
    gB1 = [cb1[min(g * GBLK + GBLK, NBLK) - 1] for g in range(ngrp)]
    gAmax = max(a1 - a0 for a0, a1 in zip(gA0, gA1))
    gBmax = max(b1 - b0 for b0, b1 in zip(gB0, gB1))
    nchmax = max(nchA[b] + nchB[b] for b in range(NBLK))

    with tile.TileContext(nc) as tc, ExitStack() as ctx:
        const = ctx.enter_context(tc.tile_pool(name="const", bufs=1))
        resid = ctx.enter_context(tc.tile_pool(name="resid", bufs=1))
        gap = ctx.enter_context(tc.tile_pool(name="gap", bufs=GBUFS))
        gbp = ctx.enter_context(tc.tile_pool(name="gbp", bufs=GBUFS))
        ohp = ctx.enter_context(tc.tile_pool(name="ohp", bufs=3))
        wk = ctx.enter_context(tc.tile_pool(name="wk", bufs=3))
        poolbig = ctx.enter_context(tc.tile_pool(name="poolbig", bufs=1))
        prepps = ctx.enter_context(tc.tile_pool(name="prepps", bufs=2, space="PSUM"))
        aggps = ctx.enter_context(tc.tile_pool(name="aggps", bufs=4, space="PSUM"))
        headps = ctx.enter_context(tc.tile_pool(name="headps", bufs=1, space="PSUM"))

        nc.gpsimd.load_library(library_config.mlp)

        def load_const(dram, shape, dt):
            t = const.tile(shape, dt, tag=dram.name)
            nc.sync.dma_start(t[:], dram[:])
            return t

        xT_t = load_const(xT_d, [2, NMAXP], BF16)
        degT_t = load_const(degT_d, [128, NBLK], BF16)
        degxT_t = load_const(degxT_d, [2, NMAXP], BF16)
        idx_t = load_const(idx_d, [128, NCHT * 8], I16)
        dstloc_t = load_const(dstloc_d, [128, DT], BF16)
        colidx_t = load_const(colidx_d, [128, 128], BF16)
        identbf_t = load_const(identbf_d, [128, 128], BF16)
        padmask_t = load_const(padmask_d, [128, NMAXP], BF16)
        gvalid_t = load_const(gvalid_d, [128, GPC], F32)
        gcnt_t = load_const(gcnt_d, [128, GPC], F32)
        W1_t = load_const(W1_d, [2, H], F32)
        W2_t = load_const(W2_d, [H, H], F32)
        W3_t = load_const(W3_d, [H, H], F32)
        Wo_t = load_const(Wo_d, [H, 2, OC], F32)
        bvec_t = load_const(bvec_d, [128, 3], F32)
        bo_t = load_const(bo_d, [GPC, OC], F32)

        rec1 = wk.tile([128, NBLK], F32, tag="rec1")
        nc.vector.reciprocal(rec1[:], degT_t[:])
        dinv_t = resid.tile([128, NBLK], F32, tag="dinv")
        nc.scalar.sqrt(dinv_t[:], rec1[:])

        rec2 = poolbig.tile([2, NMAXP], BF16, tag="big")
        with nc.allow_low_precision(reason="1/deg of small exact ints; 0.4% ok"):
            nc.vector.reciprocal(rec2[:], degxT_t[:])
        dinvxT = poolbig.tile([2, NMAXP], BF16, tag="big2")
        nc.scalar.sqrt(dinvxT[:], rec2[:])

        y0 = poolbig.tile([2, NMAXP], BF16, tag="big")
        nc.vector.tensor_tensor(y0[:], xT_t[:], dinvxT[:], OP.mult)

        dinv_rep = resid.tile([128, NMAXP], BF16, tag="dinv_rep")
        nc.gpsimd.partition_broadcast(dinv_rep[:], dinvxT[0:1, :])

        W1b = const.tile([2, H], BF16, tag="W1b")
        nc.vector.tensor_copy(W1b[:], W1_t[:])
        W2b = const.tile([128, H], BF16, tag="W2b")
        nc.vector.tensor_copy(W2b[:], W2_t[:])
        W3b = const.tile([128, H], BF16, tag="W3b")
        nc.vector.tensor_copy(W3b[:], W3_t[:])

        sbuild = resid.tile([128, NBLK, H], BF16, tag="sbuild")
        y_t = resid.tile([128, NMAXP], BF16, tag="y")

        for _rep in range(REPS):
         for L in range(3):
             Wb = (W1b, W2b, W3b)[L]
             for b0 in range(0, NBLK, PB):
                 b1 = min(b0 + PB, NBLK)
                 nb = b1 - b0
                 pp = prepps.tile([128, PB, H], F32, tag="pp")
                 for b in range(b0, b1):
                     lhs = y0[:, b * 128 : (b + 1) * 128] if L == 0 else \
                         y_t[:, b * 128 : (b + 1) * 128]
                     nc.tensor.matmul(pp[:, b - b0, :], lhs, Wb[:],
                                      start=True, stop=True)
                 if L == 0:
                     nc.vector.tensor_copy(sbuild[:, b0:b1, :], pp[:, :nb, :])
                 else:
                     nc.vector.tensor_tensor(
                         sbuild[:, b0:b1, :], pp[:, :nb, :],
                         dinv_t[:, b0:b1, None].broadcast_to((128, nb, H)),
                         OP.mult,
                     )
             nc.sync.dma_start(
                 shard_d[L].rearrange("(p b) h -> p (b h)", b=NBLK)[:, :],
                 sbuild[:].rearrange("p b h -> p (b h)"),
             )
             if "ag" not in ABLATE:
                 nc.gpsimd.collective_compute(
                     "AllGather", OP.bypass,
                     replica_groups=[list(range(NCORE))],
                     ins=[shard_d[L][:]], outs=[table_d[L][:]],
                 )

             for g in range(ngrp):
                 b0, b1 = g * GBLK, min((g + 1) * GBLK, NBLK)
                 nA = gA1[g] - gA0[g]
                 nB = gB1[g] - gB0[g]
                 need_g = "gather" not in ABLATE or "mm" not in ABLATE
                 gA = (gap.tile([128, gAmax, H], BF16, tag="gA", name="gA")
                       if nA and need_g else None)
                 gB = (gbp.tile([128, gBmax, H], BF16, tag="gB", name="gB")
                       if nB and need_g else None)
                 qg = 2 * g * QSPLIT
                 if nA and "gather" not in ABLATE:
                     for s in range(QSPLIT):
                         c0, c1 = (nA * s) // QSPLIT, (nA * (s + 1)) // QSPLIT
                         if c1 > c0:
                             nc.gpsimd.dma_gather(
                                 gA[:, c0:c1, :], table_d[L][0:HALF, :],
                                 idx_t[:, (gA0[g] + c0) * 8 : (gA0[g] + c1) * 8],
                                 (c1 - c0) * 128, (c1 - c0) * 128, H,
                                 single_packet=SINGLE_PACKET,
                                 queue_num=(qg + s) % QUEUES,
                             )
                 if nB and "gather" not in ABLATE:
                     for s in range(QSPLIT):
                         c0, c1 = (nB * s) // QSPLIT, (nB * (s + 1)) // QSPLIT
                         if c1 > c0:
                             nc.gpsimd.dma_gather(
                                 gB[:, c0:c1, :], table_d[L][HALF:, :],
                                 idx_t[:, (CA + gB0[g] + c0) * 8
                                       : (CA + gB0[g] + c1) * 8],
                                 (c1 - c0) * 128, (c1 - c0) * 128, H,
                                 single_packet=SINGLE_PACKET,
                                 queue_num=(qg + QSPLIT + s) % QUEUES,
                             )
                 for b in range(b0, b1):
                     na = nchA[b] if "mm" not in ABLATE else 0
                     nb_ = nchB[b] if "mm" not in ABLATE else 0
                     nch = na + nb_
                     if nch:
                         oh = ohp.tile([128, nchmax, 128], BF16, tag="oh",
                                       name="oh")
                         nc.vector.tensor_tensor(
                             oh[:, :nch, :],
                             colidx_t[:, None, :].broadcast_to((128, nch, 128)),
                             dstloc_t[:, dOff[b] : dOff[b] + nch, None]
                             .broadcast_to((128, nch, 128)),
                             OP.is_equal,
                         )
                     ap = aggps.tile([128, 128], F32, tag="agg")
                     for j in range(na):
                         nc.tensor.matmul(
                             ap[:], gA[:, ca0[b] - gA0[g] + j, :],
                             oh[:, j, :], start=(j == 0), stop=False,
                         )
                     for j in range(nb_):
                         nc.tensor.matmul(
                             ap[:], gB[:, cb0[b] - gB0[g] + j, :],
                             oh[:, na + j, :],
                             start=(na == 0 and j == 0), stop=False,
                         )
                     nc.tensor.matmul(
                         ap[:], sbuild[:, b, :], identbf_t[:],
                         start=(nch == 0), stop=True,
                     )
                     z = wk.tile([128, 128], F32, tag="z")
                     nc.vector.tensor_tensor(
                         z[:], ap[:], dinv_rep[:, b * 128 : (b + 1) * 128],
                         OP.mult,
                     )
                     nc.scalar.activation(
                         y_t[:, b * 128 : (b + 1) * 128], z[:], AF.Tanh,
                         bias=bvec_t[:, L : L + 1],
                     )

         gb = gblk_u * 128
         mx = resid.tile([128, GPC], F32, tag="mx")
         sm = resid.tile([128, GPC], F32, tag="sm")
         for g in range(GPC):
             zg = wk.tile([128, gb], F32, tag="zg")
             nc.vector.tensor_scalar(
                 zg[:], y_t[:, g * gb : (g + 1) * gb], 2.0, None, OP.add)
             nc.vector.tensor_tensor(
                 zg[:], zg[:], padmask_t[:, g * gb : (g + 1) * gb], OP.mult)
             nc.vector.tensor_reduce(
                 mx[:, g : g + 1], zg[:], mybir.AxisListType.X, OP.max)
             nc.vector.tensor_reduce(
                 sm[:, g : g + 1], zg[:], mybir.AxisListType.X, OP.add)
         recg = wk.tile([128, GPC], F32, tag="recg")
         nc.vector.reciprocal(recg[:], gcnt_t[:])
         mean2 = resid.tile([128, GPC], F32, tag="mean2")
         nc.vector.tensor_tensor(mean2[:], sm[:], recg[:], OP.mult)
         nc.vector.tensor_scalar(mean2[:], mean2[:], 2.0, None, OP.subtract)
         nc.vector.tensor_tensor(mean2[:], mean2[:], gvalid_t[:], OP.mult)
         mx2 = resid.tile([128, GPC], F32, tag="mx2")
         nc.vector.tensor_scalar(mx2[:], mx[:], 2.0, None, OP.subtract)
         nc.vector.tensor_tensor(mx2[:], mx2[:], gvalid_t[:], OP.mult)

         headp = headps.tile([GPC, OC], F32, tag="head")
         nc.tensor.matmul(headp[:], mx2[:], Wo_t[:, 0, :], start=True, stop=False)
         nc.tensor.matmul(headp[:], mean2[:], Wo_t[:, 1, :], start=False, stop=True)
         hsum = wk.tile([GPC, OC], F32, tag="hsum")
         nc.vector.tensor_tensor(hsum[:], headp[:], bo_t[:], OP.add)
         ofin = wk.tile([GPC, OC], F32, tag="ofin")
         nc.scalar.activation(ofin[:], hsum[:], AF.Tanh)
         nc.sync.dma_start(out_d[:], ofin[:])

    nc.compile()
    return nc


def make_in_maps(meta, inputs):
    colidx = np.tile(np.arange(128, dtype=np.float32), (128, 1)).astype(
        ml_dtypes.bfloat16)
    identbf = np.eye(128, dtype=np.float32).astype(ml_dtypes.bfloat16)
    bvec = np.stack(
        [np.asarray(inputs[b], np.float32) for b in ("b1", "b2", "b3")], 1)
    bo_t = np.tile(np.asarray(inputs["bo"], np.float32), (GPC, 1))
    Wo = np.asarray(inputs["Wo"], np.float32)
    Wo_t = np.ascontiguousarray(np.stack([Wo[:H], Wo[H:]], axis=1))
    gsz = meta["gsz"]
    maps = []
    for k, c in enumerate(meta["cores"]):
        gcnt = np.maximum(gsz[k * GPC : (k + 1) * GPC].astype(np.float32), 1.0)
        maps.append({
            "xT": np.asarray(c["xT"]),
            "degT": np.asarray(c["degT"]).astype(ml_dtypes.bfloat16),
            "degxT": np.asarray(c["degxT"]).astype(ml_dtypes.bfloat16),
            "idx": np.asarray(c["idx"]),
            "dstloc": np.asarray(c["dstloc"]),
            "colidx": colidx,
            "identbf": identbf,
            "padmask": np.asarray(c["padmask"]),
            "gvalid": np.asarray(c["gvalid"]),
            "gcnt": np.tile(gcnt[None, :], (128, 1)).astype(np.float32),
            "W1": np.asarray(inputs["W1"], np.float32),
            "W2": np.asarray(inputs["W2"], np.float32),
            "W3": np.asarray(inputs["W3"], np.float32),
            "Wo": Wo_t,
            "bvec": bvec.astype(np.float32),
            "bo": bo_t,
        })
    return maps


_CACHE = {}


def kernel(x, edge_index, batch, W1, b1, W2, b2, W3, b3, Wo, bo):
    x = np.asarray(x, np.float32)
    meta = prep(x, np.asarray(edge_index), np.asarray(batch), 64)
    key = (meta["NBLK"], meta["CA"], meta["CB"], tuple(meta["dOff"]))
    if key not in _CACHE:
        _CACHE[key] = build(meta)
    nc = _CACHE[key]
    inputs = dict(W1=W1, b1=b1, W2=W2, b2=b2, W3=W3, b3=b3, Wo=Wo, bo=bo)
    in_maps = make_in_maps(meta, inputs)
    res = run_bass_kernel_spmd(nc, in_maps, core_ids=list(range(8)), trace=False)
    out = np.concatenate([res.results[k]["out"] for k in range(8)], 0)
    return np.ascontiguousarray(out, dtype=np.float32)


# revision 6
# speedup vs baseline: 1.2926x; 1.0320x over previous
"""Self-contained Trainium2 Bass kernel for nn_GCNMagnetModel
(3-layer GCN: N=50000 nodes, E=600000 edges, H=128, 64 graphs,
8 NeuronCores, SPMD single NEFF).

Sharding: nodes/edges sharded by graph id (graphs 8k..8k+7 -> core k), so
segment pools are core-local; weight matrices replicated. Per layer:
bf16 table t1=(h@W)*dinv[src] -> shard AllGather (HBM Shared) -> per
dst-block dma_gather of edge src rows + one-hot matmuls accumulate
agg[feature,dst] on the PE -> epilogue *dinv[dst] + tanh. Pad-trimmed
gather streams:

Edge streams per (half A/B) use EXACT per-(dst-block) counts padded only to
the max over cores (not rounded to 128). Chunk boundaries no longer align
with dst-block boundaries: a boundary chunk is gathered once but appears in
both adjacent blocks' one-hot matmuls, with the out-of-block lanes masked
to -1 in that block's dstloc columns (is_equal never matches -> zero
column). Cuts gather descriptors ~11-15% at the cost of ~1 extra matmul
per block boundary.
"""
import numpy as np
import ml_dtypes
from contextlib import ExitStack

import concourse.tile as tile
import concourse.mybir as mybir
from concourse import bacc
from concourse import library_config
from concourse.bass_utils import run_bass_kernel_spmd

NCORE = 8
P = 128
GPC = 8
H = 128
OC = 41

F32 = mybir.dt.float32
BF16 = mybir.dt.bfloat16
I16 = mybir.dt.int16
AF = mybir.ActivationFunctionType
OP = mybir.AluOpType


def wrap16(v):  # [n] -> [128, n/16]: idx[i%16, i//16] tiled 8x
    a = v.reshape(-1, 16).T
    return np.tile(a, (8, 1)).copy()


def prep(x, edge_index, batch, n_graphs=64):
    N = x.shape[0]
    batch = np.asarray(batch)
    src_g, dst_g = np.asarray(edge_index[0]), np.asarray(edge_index[1])
    E = src_g.shape[0]

    gstart = np.searchsorted(batch, np.arange(n_graphs), side="left")
    gend = np.searchsorted(batch, np.arange(n_graphs), side="right")
    gsz = gend - gstart

    gblk_u = max(int((gsz.max() + P - 1) // P), 1)
    NBLK = GPC * gblk_u
    NMAXP = NBLK * P
    HALF = 4 * NMAXP
    assert HALF < 32768

    node_core = batch // GPC
    node_loc = (batch % GPC) * (gblk_u * P) + (np.arange(N) - gstart[batch])
    node_row = node_core * NMAXP + (node_loc % P) * NBLK + node_loc // P

    deg = np.bincount(dst_g, minlength=N).astype(np.float32) + 1.0

    e_core = node_core[dst_g]
    e_dstloc = node_loc[dst_g]
    e_blk = e_dstloc // P
    e_dl = e_dstloc % P
    e_row = node_row[src_g]
    e_half = (e_row >= HALF).astype(np.int64)

    cnts = np.zeros((NCORE, NBLK, 2), np.int64)
    np.add.at(cnts, (e_core, e_blk, e_half), 1)
    mA = cnts[:, :, 0].max(axis=0)     # exact max-over-core counts
    mB = cnts[:, :, 1].max(axis=0)
    eoffA = np.r_[0, np.cumsum(mA)]    # edge-unit offsets
    eoffB = np.r_[0, np.cumsum(mB)]
    EA, EB = int(eoffA[-1]), int(eoffB[-1])
    CA, CB = (EA + P - 1) // P, (EB + P - 1) // P
    # chunk ranges per block (may share boundary chunks)
    ca0 = (eoffA[:-1] // P).astype(np.int64)
    ca1 = ((eoffA[1:] + P - 1) // P).astype(np.int64)
    cb0 = (eoffB[:-1] // P).astype(np.int64)
    cb1 = ((eoffB[1:] + P - 1) // P).astype(np.int64)
    nchA = np.where(mA > 0, ca1 - ca0, 0)
    nchB = np.where(mB > 0, cb1 - cb0, 0)
    nch = nchA + nchB
    dOff = np.r_[0, np.cumsum(nch)]    # per-block dstloc column offsets
    DT = int(dOff[-1])

    # per-core edge slot assignment at exact offsets
    order = np.lexsort((e_row, e_half, e_blk, e_core))
    so_core, so_blk, so_half = e_core[order], e_blk[order], e_half[order]
    so_row, so_dl = e_row[order], e_dl[order]
    key = (so_core * NBLK + so_blk) * 2 + so_half
    runstart = np.r_[0, np.flatnonzero(np.diff(key)) + 1]
    runid = np.zeros(E, np.int64)
    runid[runstart[1:]] = 1
    runid = np.cumsum(runid)
    pos_in_run = np.arange(E) - runstart[runid]

    slotA = eoffA[so_blk] + pos_in_run
    slotB = eoffB[so_blk] + pos_in_run
    slot = np.where(so_half == 0, slotA, CA * P + slotB)
    idxv = np.where(so_half == 0, so_row, so_row - HALF).astype(np.int16)

    NCHT = CA + CB
    idx_all = np.zeros((NCORE, NCHT * P), np.int16)
    dl_all = np.full((NCORE, NCHT * P), -1.0, np.float32)
    sl_full = np.where(so_half == 0, slotA, EA + slotB)  # for stream dl
    # idx stream: A region [0, CA*P), B region [CA*P, ...)
    idx_all[so_core, slot] = idxv

    cores = []
    for k in range(NCORE):
        sel = node_core == k
        loc_k = node_loc[sel]
        xk = np.zeros((NMAXP, 2), np.float32)
        xk[loc_k] = np.asarray(x)[sel]
        degk = np.ones(NMAXP, np.float32)
        degk[loc_k] = deg[sel]
        degT = np.ones((P, NBLK), np.float32)
        degT[loc_k % P, loc_k // P] = deg[sel]
        padmask = np.zeros(NMAXP, np.float32)
        padmask[loc_k] = 1.0

        # dl streams (exact positions) for this core
        dlA = np.full(CA * P, -1.0, np.float32)
        dlB = np.full(CB * P, -1.0, np.float32)
        mk = so_core == k
        a = mk & (so_half == 0)
        b = mk & (so_half == 1)
        dlA[slotA[a]] = so_dl[a]
        dlB[slotB[b]] = so_dl[b]

        # per-block dstloc columns with out-of-block lanes masked to -1
        dcols = np.full((DT, P), -1.0, np.float32)
        e_idx = np.arange(P)
        for bb in range(NBLK):
            col = int(dOff[bb])
            if mA[bb] > 0:
                for j in range(int(nchA[bb])):
                    c = int(ca0[bb]) + j
                    seg = dlA[c * P : (c + 1) * P].copy()
                    epos = c * P + e_idx
                    seg[(epos < eoffA[bb]) | (epos >= eoffA[bb + 1])] = -1.0
                    dcols[col + j] = seg
                col += int(nchA[bb])
            if mB[bb] > 0:
                for j in range(int(nchB[bb])):
                    c = int(cb0[bb]) + j
                    seg = dlB[c * P : (c + 1) * P].copy()
                    epos = c * P + e_idx
                    seg[(epos < eoffB[bb]) | (epos >= eoffB[bb + 1])] = -1.0
                    dcols[col + j] = seg

        gv = (gsz[k * GPC : (k + 1) * GPC] > 0).astype(np.float32)
        cores.append(
            dict(
                xT=np.ascontiguousarray(xk.T).astype(ml_dtypes.bfloat16),
                degT=degT,
                degxT=np.tile(degk[None, :], (2, 1)).astype(np.float32),
                idx=wrap16(idx_all[k]),
                dstloc=np.ascontiguousarray(dcols.T).astype(ml_dtypes.bfloat16),
                padmask=np.tile(padmask[None, :], (P, 1)).astype(ml_dtypes.bfloat16),
                gvalid=np.tile(gv[None, :], (P, 1)).astype(np.float32),
            )
        )

    return dict(
        NBLK=NBLK, NMAXP=NMAXP, HALF=HALF, gblk_u=gblk_u,
        mA=mA, mB=mB, eoffA=eoffA, eoffB=eoffB, EA=EA, EB=EB,
        CA=CA, CB=CB, ca0=ca0, ca1=ca1, cb0=cb0, cb1=cb1,
        nchA=nchA, nchB=nchB, nch=nch, dOff=dOff, DT=DT, NCHT=NCHT,
        gsz=gsz, cores=cores, deg=deg,
    )


def build(meta, GBLK=4, SINGLE_PACKET=False, ABLATE=(), REPS=1, QUEUES=4,
          GBUFS=3, QSPLIT=2):
    NBLK, NMAXP, HALF = meta["NBLK"], meta["NMAXP"], meta["HALF"]
    CA, CB, NCHT, DT = meta["CA"], meta["CB"], meta["NCHT"], meta["DT"]
    ca0 = [int(v) for v in meta["ca0"]]
    ca1 = [int(v) for v in meta["ca1"]]
    cb0 = [int(v) for v in meta["cb0"]]
    cb1 = [int(v) for v in meta["cb1"]]
    nchA = [int(v) for v in meta["nchA"]]
    nchB = [int(v) for v in meta["nchB"]]
    dOff = [int(v) for v in meta["dOff"]]
    gblk_u = meta["gblk_u"]
    NTAB = NCORE * NMAXP
    ngrp = (NBLK + GBLK - 1) // GBLK

    nc = bacc.Bacc(None, target_bir_lowering=False, num_swdge_queues=QUEUES)

    xT_d = nc.dram_tensor("xT", [2, NMAXP], BF16, kind="ExternalInput")
    degT_d = nc.dram_tensor("degT", [128, NBLK], BF16, kind="ExternalInput")
    degxT_d = nc.dram_tensor("degxT", [2, NMAXP], BF16, kind="ExternalInput")
    idx_d = nc.dram_tensor("idx", [128, NCHT * 8], I16, kind="ExternalInput")
    dstloc_d = nc.dram_tensor("dstloc", [128, DT], BF16, kind="ExternalInput")
    colidx_d = nc.dram_tensor("colidx", [128, 128], BF16, kind="ExternalInput")
    identbf_d = nc.dram_tensor("identbf", [128, 128], BF16, kind="ExternalInput")
    padmask_d = nc.dram_tensor("padmask", [128, NMAXP], BF16, kind="ExternalInput")
    gvalid_d = nc.dram_tensor("gvalid", [128, GPC], F32, kind="ExternalInput")
    gcnt_d = nc.dram_tensor("gcnt", [128, GPC], F32, kind="ExternalInput")
    W1_d = nc.dram_tensor("W1", [2, H], F32, kind="ExternalInput")
    W2_d = nc.dram_tensor("W2", [H, H], F32, kind="ExternalInput")
    W3_d = nc.dram_tensor("W3", [H, H], F32, kind="ExternalInput")
    Wo_d = nc.dram_tensor("Wo", [H, 2, OC], F32, kind="ExternalInput")
    bvec_d = nc.dram_tensor("bvec", [128, 3], F32, kind="ExternalInput")
    bo_d = nc.dram_tensor("bo", [GPC, OC], F32, kind="ExternalInput")
    out_d = nc.dram_tensor("out", [GPC, OC], F32, kind="ExternalOutput")

    shard_d = [nc.dram_tensor(f"shard{L}", [NMAXP, H], BF16) for L in range(3)]
    table_d = [
        nc.dram_tensor(f"table{L}", [NTAB, H], BF16, addr_space="Shared")
        for L in range(3)
    ]

    PB = 4
    # group gather chunk ranges
    gA0 = [ca0[g * GBLK] for g in range(ngrp)]
    gA1 = [ca1[min(g * GBLK + GBLK, NBLK) - 1] for g in range(ngrp)]
    gB0 = [cb0[g * GBLK] for g in range(ngrp)]
    gB1 = [cb1[min(g * GBLK + GBLK, NBLK) - 1] for g in range(ngrp)]
    gAmax = max(a1 - a0 for a0, a1 in zip(gA0, gA1))
    gBmax = max(b1 - b0 for b0, b1 in zip(gB0, gB1))
    nchmax = max(nchA[b] + nchB[b] for b in range(NBLK))

    with tile.TileContext(nc) as tc, ExitStack() as ctx:
        const = ctx.enter_context(tc.tile_pool(name="const", bufs=1))
        resid = ctx.enter_context(tc.tile_pool(name="resid", bufs=1))
        gap = ctx.enter_context(tc.tile_pool(name="gap", bufs=GBUFS))
        gbp = ctx.enter_context(tc.tile_pool(name="gbp", bufs=GBUFS))
        ohp = ctx.enter_context(tc.tile_pool(name="ohp", bufs=3))
        wk = ctx.enter_context(tc.tile_pool(name="wk", bufs=3))
        poolbig = ctx.enter_context(tc.tile_pool(name="poolbig", bufs=1))
        prepps = ctx.enter_context(tc.tile_pool(name="prepps", bufs=2, space="PSUM"))
        aggps = ctx.enter_context(tc.tile_pool(name="aggps", bufs=4, space="PSUM"))
        headps = ctx.enter_context(tc.tile_pool(name="headps", bufs=1, space="PSUM"))

        nc.gpsimd.load_library(library_config.mlp)

        def load_const(dram, shape, dt):
            t = const.tile(shape, dt, tag=dram.name)
            nc.sync.dma_start(t[:], dram[:])
            return t

        xT_t = load_const(xT_d, [2, NMAXP], BF16)
        degT_t = load_const(degT_d, [128, NBLK], BF16)
        degxT_t = load_const(degxT_d, [2, NMAXP], BF16)
        idx_t = load_const(idx_d, [128, NCHT * 8], I16)
        dstloc_t = load_const(dstloc_d, [128, DT], BF16)
        colidx_t = load_const(colidx_d, [128, 128], BF16)
        identbf_t = load_const(identbf_d, [128, 128], BF16)
        padmask_t = load_const(padmask_d, [128, NMAXP], BF16)
        gvalid_t = load_const(gvalid_d, [128, GPC], F32)
        gcnt_t = load_const(gcnt_d, [128, GPC], F32)
        W1_t = load_const(W1_d, [2, H], F32)
        W2_t = load_const(W2_d, [H, H], F32)
        W3_t = load_const(W3_d, [H, H], F32)
        Wo_t = load_const(Wo_d, [H, 2, OC], F32)
        bvec_t = load_const(bvec_d, [128, 3], F32)
        bo_t = load_const(bo_d, [GPC, OC], F32)

        rec1 = wk.tile([128, NBLK], F32, tag="rec1")
        nc.vector.reciprocal(rec1[:], degT_t[:])
        dinv_t = resid.tile([128, NBLK], F32, tag="dinv")
        nc.scalar.sqrt(dinv_t[:], rec1[:])

        rec2 = poolbig.tile([2, NMAXP], BF16, tag="big")
        with nc.allow_low_precision(reason="1/deg of small exact ints; 0.4% ok"):
            nc.vector.reciprocal(rec2[:], degxT_t[:])
        dinvxT = poolbig.tile([2, NMAXP], BF16, tag="big2")
        nc.scalar.sqrt(dinvxT[:], rec2[:])

        y0 = poolbig.tile([2, NMAXP], BF16, tag="big")
        nc.vector.tensor_tensor(y0[:], xT_t[:], dinvxT[:], OP.mult)

        dinv_rep = resid.tile([128, NMAXP], BF16, tag="dinv_rep")
        nc.gpsimd.partition_broadcast(dinv_rep[:], dinvxT[0:1, :])

        W1b = const.tile([2, H], BF16, tag="W1b")
        nc.vector.tensor_copy(W1b[:], W1_t[:])
        W2b = const.tile([128, H], BF16, tag="W2b")
        nc.vector.tensor_copy(W2b[:], W2_t[:])
        W3b = const.tile([128, H], BF16, tag="W3b")
        nc.vector.tensor_copy(W3b[:], W3_t[:])

        sbuild = resid.tile([128, NBLK, H], BF16, tag="sbuild")
        y_t = resid.tile([128, NMAXP], BF16, tag="y")

        for _rep in range(REPS):
         for L in range(3):
             Wb = (W1b, W2b, W3b)[L]
             for b0 in range(0, NBLK, PB):
                 b1 = min(b0 + PB, NBLK)
                 nb = b1 - b0
                 pp = prepps.tile([128, PB, H], F32, tag="pp")
                 for b in range(b0, b1):
                     lhs = y0[:, b * 128 : (b + 1) * 128] if L == 0 else \
                         y_t[:, b * 128 : (b + 1) * 128]
                     nc.tensor.matmul(pp[:, b - b0, :], lhs, Wb[:],
                                      start=True, stop=True)
                 if L == 0:
                     nc.vector.tensor_copy(sbuild[:, b0:b1, :], pp[:, :nb, :])
                 else:
                     nc.vector.tensor_tensor(
                         sbuild[:, b0:b1, :], pp[:, :nb, :],
                         dinv_t[:, b0:b1, None].broadcast_to((128, nb, H)),
                         OP.mult,
                     )
             nc.sync.dma_start(
                 shard_d[L].rearrange("(p b) h -> p (b h)", b=NBLK)[:, :],
                 sbuild[:].rearrange("p b h -> p (b h)"),
             )
             if "ag" not in ABLATE:
                 nc.gpsimd.collective_compute(
                     "AllGather", OP.bypass,
                     replica_groups=[list(range(NCORE))],
                     ins=[shard_d[L][:]], outs=[table_d[L][:]],
                 )

             for g in range(ngrp):
                 b0, b1 = g * GBLK, min((g + 1) * GBLK, NBLK)
                 nA = gA1[g] - gA0[g]
                 nB = gB1[g] - gB0[g]
                 need_g = "gather" not in ABLATE or "mm" not in ABLATE
                 gA = (gap.tile([128, gAmax, H], BF16, tag="gA", name="gA")
                       if nA and need_g else None)
                 gB = (gbp.tile([128, gBmax, H], BF16, tag="gB", name="gB")
                       if nB and need_g else None)
                 qg = 2 * g * QSPLIT
                 if nA and "gather" not in ABLATE:
                     for s in range(QSPLIT):
                         c0, c1 = (nA * s) // QSPLIT, (nA * (s + 1)) // QSPLIT
                         if c1 > c0:
                             nc.gpsimd.dma_gather(
                                 gA[:, c0:c1, :], table_d[L][0:HALF, :],
                                 idx_t[:, (gA0[g] + c0) * 8 : (gA0[g] + c1) * 8],
                                 (c1 - c0) * 128, (c1 - c0) * 128, H,
                                 single_packet=SINGLE_PACKET,
                                 queue_num=(qg + s) % QUEUES,
                             )
                 if nB and "gather" not in ABLATE:
                     for s in range(QSPLIT):
                         c0, c1 = (nB * s) // QSPLIT, (nB * (s + 1)) // QSPLIT
                         if c1 > c0:
                             nc.gpsimd.dma_gather(
                                 gB[:, c0:c1, :], table_d[L][HALF:, :],
                                 idx_t[:, (CA + gB0[g] + c0) * 8
                                       : (CA + gB0[g] + c1) * 8],
                                 (c1 - c0) * 128, (c1 - c0) * 128, H,
                                 single_packet=SINGLE_PACKET,
                                 queue_num=(qg + QSPLIT + s) % QUEUES,
                             )
                 for b in range(b0, b1):
                     na = nchA[b] if "mm" not in ABLATE else 0
                     nb_ = nchB[b] if "mm" not in ABLATE else 0
                     nch = na + nb_
                     if nch:
                         oh = ohp.tile([128, nchmax, 128], BF16, tag="oh",
                                       name="oh")
                         nc.vector.tensor_tensor(
                             oh[:, :nch, :],
                             colidx_t[:, None, :].broadcast_to((128, nch, 128)),
                             dstloc_t[:, dOff[b] : dOff[b] + nch, None]
                             .broadcast_to((128, nch, 128)),
                             OP.is_equal,
                         )
                     ap = aggps.tile([128, 128], F32, tag="agg")
                     for j in range(na):
                         nc.tensor.matmul(
                             ap[:], gA[:, ca0[b] - gA0[g] + j, :],
                             oh[:, j, :], start=(j == 0), stop=False,
                         )
                     for j in range(nb_):
                         nc.tensor.matmul(
                             ap[:], gB[:, cb0[b] - gB0[g] + j, :],
                             oh[:, na + j, :],
                             start=(na == 0 and j == 0), stop=False,
                         )
                     nc.tensor.matmul(
                         ap[:], sbuild[:, b, :], identbf_t[:],
                         start=(nch == 0), stop=True,
                     )
                     z = wk.tile([128, 128], F32, tag="z")
                     nc.vector.tensor_tensor(
                         z[:], ap[:], dinv_rep[:, b * 128 : (b + 1) * 128],
                         OP.mult,
                     )
                     nc.scalar.activation(
                         y_t[:, b * 128 : (b + 1) * 128], z[:], AF.Tanh,
                         bias=bvec_t[:, L : L + 1],
                     )

         gb = gblk_u * 128
         mx = resid.tile([128, GPC], F32, tag="mx")
         sm = resid.tile([128, GPC], F32, tag="sm")
         for g in range(GPC):
             zg = wk.tile([128, gb], F32, tag="zg")
             nc.vector.tensor_scalar(
                 zg[:], y_t[:, g * gb : (g + 1) * gb], 2.0, None, OP.add)
             nc.vector.tensor_tensor(
                 zg[:], zg[:], padmask_t[:, g * gb : (g + 1) * gb], OP.mult)
             nc.vector.tensor_reduce(
                 mx[:, g : g + 1], zg[:], mybir.AxisListType.X, OP.max)
             nc.vector.tensor_reduce(
                 sm[:, g : g + 1], zg[:], mybir.AxisListType.X, OP.add)
         recg = wk.tile([128, GPC], F32, tag="recg")
         nc.vector.reciprocal(recg[:], gcnt_t[:])
         mean2 = resid.tile([128, GPC], F32, tag="mean2")
         nc.vector.tensor_tensor(mean2[:], sm[:], recg[:], OP.mult)
         nc.vector.tensor_scalar(mean2[:], mean2[:], 2.0, None, OP.subtract)
         nc.vector.tensor_tensor(mean2[:], mean2[:], gvalid_t[:], OP.mult)
         mx2 = resid.tile([128, GPC], F32, tag="mx2")
         nc.vector.tensor_scalar(mx2[:], mx[:], 2.0, None, OP.subtract)
         nc.vector.tensor_tensor(mx2[:], mx2[:], gvalid_t[:], OP.mult)

         headp = headps.tile([GPC, OC], F32, tag="head")
         nc.tensor.matmul(headp[:], mx2[:], Wo_t[:, 0, :], start=True, stop=False)
         nc.tensor.matmul(headp[:], mean2[:], Wo_t[:, 1, :], start=False, stop=True)
         hsum = wk.tile([GPC, OC], F32, tag="hsum")
         nc.vector.tensor_tensor(hsum[:], headp[:], bo_t[:], OP.add)
         ofin = wk.tile([GPC, OC], F32, tag="ofin")
         nc.scalar.activation(ofin[:], hsum[:], AF.Tanh)
         nc.sync.dma_start(out_d[:], ofin[:])

    nc.compile()
    return nc


def make_in_maps(meta, inputs):
    colidx = np.tile(np.arange(128, dtype=np.float32), (128, 1)).astype(
        ml_dtypes.bfloat16)
    identbf = np.eye(128, dtype=np.float32).astype(ml_dtypes.bfloat16)
    bvec = np.stack(
        [np.asarray(inputs[b], np.float32) for b in ("b1", "b2", "b3")], 1)
    bo_t = np.tile(np.asarray(inputs["bo"], np.float32), (GPC, 1))
    Wo = np.asarray(inputs["Wo"], np.float32)
    Wo_t = np.ascontiguousarray(np.stack([Wo[:H], Wo[H:]], axis=1))
    gsz = meta["gsz"]
    maps = []
    for k, c in enumerate(meta["cores"]):
        gcnt = np.maximum(gsz[k * GPC : (k + 1) * GPC].astype(np.float32), 1.0)
        maps.append({
            "xT": np.asarray(c["xT"]),
            "degT": np.asarray(c["degT"]).astype(ml_dtypes.bfloat16),
            "degxT": np.asarray(c["degxT"]).astype(ml_dtypes.bfloat16),
            "idx": np.asarray(c["idx"]),
            "dstloc": np.asarray(c["dstloc"]),
            "colidx": colidx,
            "identbf": identbf,
            "padmask": np.asarray(c["padmask"]),
            "gvalid": np.asarray(c["gvalid"]),
            "gcnt": np.tile(gcnt[None, :], (128, 1)).astype(np.float32),
            "W1": np.asarray(inputs["W1"], np.float32),
            "W2": np.asarray(inputs["W2"], np.float32),
            "W3": np.asarray(inputs["W3"], np.float32),
            "Wo": Wo_t,
            "bvec": bvec.astype(np.float32),
            "bo": bo_t,
        })
    return maps


_CACHE = {}


def kernel(x, edge_index, batch, W1, b1, W2, b2, W3, b3, Wo, bo):
    x = np.asarray(x, np.float32)
    meta = prep(x, np.asarray(edge_index), np.asarray(batch), 64)
    key = (meta["NBLK"], meta["CA"], meta["CB"], tuple(meta["dOff"]))
    if key not in _CACHE:
        _CACHE[key] = build(meta)
    nc = _CACHE[key]
    inputs = dict(W1=W1, b1=b1, W2=W2, b2=b2, W3=W3, b3=b3, Wo=Wo, bo=bo)
    in_maps = make_in_maps(meta, inputs)
    res = run_bass_kernel_spmd(nc, in_maps, core_ids=list(range(8)), trace=False)
    out = np.concatenate([res.results[k]["out"] for k in range(8)], 0)
    return np.ascontiguousarray(out, dtype=np.float32)


# revision 8
# speedup vs baseline: 1.4105x; 1.0912x over previous
"""Self-contained Trainium2 Bass kernel for nn_GCNMagnetModel
(3-layer GCN: N=50000 nodes, E=600000 edges, H=128, 64 graphs,
8 NeuronCores, SPMD single NEFF).

Sharding: nodes/edges sharded by graph id (graphs 8k..8k+7 -> core k), so
segment pools are core-local; weight matrices replicated. Per layer:
bf16 table t1=(h@W)*dinv[src] -> shard AllGather (HBM Shared) -> per
dst-block dma_gather of edge src rows + one-hot matmuls accumulate
agg[feature,dst] on the PE -> epilogue *dinv[dst] + tanh. Pad-trimmed
gather streams:

Edge streams per (half A/B) use EXACT per-(dst-block) counts padded only to
the max over cores (not rounded to 128). Chunk boundaries no longer align
with dst-block boundaries: a boundary chunk is gathered once but appears in
both adjacent blocks' one-hot matmuls, with the out-of-block lanes masked
to -1 in that block's dstloc columns (is_equal never matches -> zero
column). Cuts gather descriptors ~11-15% at the cost of ~1 extra matmul
per block boundary.
"""
import numpy as np
import ml_dtypes
from contextlib import ExitStack

import concourse.tile as tile
import concourse.mybir as mybir
from concourse import bacc
from concourse import library_config
from concourse.bass_utils import run_bass_kernel_spmd

NCORE = 8
P = 128
GPC = 8
H = 128
OC = 41

F32 = mybir.dt.float32
BF16 = mybir.dt.bfloat16
I16 = mybir.dt.int16
AF = mybir.ActivationFunctionType
OP = mybir.AluOpType


def wrap16(v):  # [n] -> [128, n/16]: idx[i%16, i//16] tiled 8x
    a = v.reshape(-1, 16).T
    return np.tile(a, (8, 1)).copy()


def prep(x, edge_index, batch, n_graphs=64):
    N = x.shape[0]
    batch = np.asarray(batch)
    src_g, dst_g = np.asarray(edge_index[0]), np.asarray(edge_index[1])
    E = src_g.shape[0]

    gstart = np.searchsorted(batch, np.arange(n_graphs), side="left")
    gend = np.searchsorted(batch, np.arange(n_graphs), side="right")
    gsz = gend - gstart

    gblk_u = max(int((gsz.max() + P - 1) // P), 1)
    NBLK = GPC * gblk_u
    NMAXP = NBLK * P
    HALF = 4 * NMAXP
    assert HALF < 32768

    node_core = batch // GPC
    node_loc = (batch % GPC) * (gblk_u * P) + (np.arange(N) - gstart[batch])
    node_row = node_core * NMAXP + (node_loc % P) * NBLK + node_loc // P

    deg = np.bincount(dst_g, minlength=N).astype(np.float32) + 1.0

    e_core = node_core[dst_g]
    e_dstloc = node_loc[dst_g]
    e_blk = e_dstloc // P
    e_dl = e_dstloc % P
    e_row = node_row[src_g]
    e_half = (e_row >= HALF).astype(np.int64)

    cnts = np.zeros((NCORE, NBLK, 2), np.int64)
    np.add.at(cnts, (e_core, e_blk, e_half), 1)
    mA = cnts[:, :, 0].max(axis=0)     # exact max-over-core counts
    mB = cnts[:, :, 1].max(axis=0)
    eoffA = np.r_[0, np.cumsum(mA)]    # edge-unit offsets
    eoffB = np.r_[0, np.cumsum(mB)]
    EA, EB = int(eoffA[-1]), int(eoffB[-1])
    CA, CB = (EA + P - 1) // P, (EB + P - 1) // P
    # chunk ranges per block (may share boundary chunks)
    ca0 = (eoffA[:-1] // P).astype(np.int64)
    ca1 = ((eoffA[1:] + P - 1) // P).astype(np.int64)
    cb0 = (eoffB[:-1] // P).astype(np.int64)
    cb1 = ((eoffB[1:] + P - 1) // P).astype(np.int64)
    nchA = np.where(mA > 0, ca1 - ca0, 0)
    nchB = np.where(mB > 0, cb1 - cb0, 0)
    nch = nchA + nchB
    dOff = np.r_[0, np.cumsum(nch)]    # per-block dstloc column offsets
    DT = int(dOff[-1])

    # per-core edge slot assignment at exact offsets
    order = np.lexsort((e_row, e_half, e_blk, e_core))
    so_core, so_blk, so_half = e_core[order], e_blk[order], e_half[order]
    so_row, so_dl = e_row[order], e_dl[order]
    key = (so_core * NBLK + so_blk) * 2 + so_half
    runstart = np.r_[0, np.flatnonzero(np.diff(key)) + 1]
    runid = np.zeros(E, np.int64)
    runid[runstart[1:]] = 1
    runid = np.cumsum(runid)
    pos_in_run = np.arange(E) - runstart[runid]

    slotA = eoffA[so_blk] + pos_in_run
    slotB = eoffB[so_blk] + pos_in_run
    slot = np.where(so_half == 0, slotA, CA * P + slotB)
    idxv = np.where(so_half == 0, so_row, so_row - HALF).astype(np.int16)

    NCHT = CA + CB
    idx_all = np.zeros((NCORE, NCHT * P), np.int16)
    dl_all = np.full((NCORE, NCHT * P), -1.0, np.float32)
    sl_full = np.where(so_half == 0, slotA, EA + slotB)  # for stream dl
    # idx stream: A region [0, CA*P), B region [CA*P, ...)
    idx_all[so_core, slot] = idxv

    cores = []
    for k in range(NCORE):
        sel = node_core == k
        loc_k = node_loc[sel]
        xk = np.zeros((NMAXP, 2), np.float32)
        xk[loc_k] = np.asarray(x)[sel]
        degk = np.ones(NMAXP, np.float32)
        degk[loc_k] = deg[sel]
        degT = np.ones((P, NBLK), np.float32)
        degT[loc_k % P, loc_k // P] = deg[sel]
        padmask = np.zeros(NMAXP, np.float32)
        padmask[loc_k] = 1.0

        # dl streams (exact positions) for this core
        dlA = np.full(CA * P, -1.0, np.float32)
        dlB = np.full(CB * P, -1.0, np.float32)
        mk = so_core == k
        a = mk & (so_half == 0)
        b = mk & (so_half == 1)
        dlA[slotA[a]] = so_dl[a]
        dlB[slotB[b]] = so_dl[b]

        # per-block dstloc columns with out-of-block lanes masked to -1
        dcols = np.full((DT, P), -1.0, np.float32)
        e_idx = np.arange(P)
        for bb in range(NBLK):
            col = int(dOff[bb])
            if mA[bb] > 0:
                for j in range(int(nchA[bb])):
                    c = int(ca0[bb]) + j
                    seg = dlA[c * P : (c + 1) * P].copy()
                    epos = c * P + e_idx
                    seg[(epos < eoffA[bb]) | (epos >= eoffA[bb + 1])] = -1.0
                    dcols[col + j] = seg
                col += int(nchA[bb])
            if mB[bb] > 0:
                for j in range(int(nchB[bb])):
                    c = int(cb0[bb]) + j
                    seg = dlB[c * P : (c + 1) * P].copy()
                    epos = c * P + e_idx
                    seg[(epos < eoffB[bb]) | (epos >= eoffB[bb + 1])] = -1.0
                    dcols[col + j] = seg

        gv = (gsz[k * GPC : (k + 1) * GPC] > 0).astype(np.float32)
        cores.append(
            dict(
                xT=np.ascontiguousarray(xk.T).astype(ml_dtypes.bfloat16),
                degT=degT,
                degxT=np.tile(degk[None, :], (2, 1)).astype(np.float32),
                idx=wrap16(idx_all[k]),
                dstloc=np.ascontiguousarray(dcols.T).astype(ml_dtypes.bfloat16),
                padmask=np.tile(padmask[None, :], (P, 1)).astype(ml_dtypes.bfloat16),
                gvalid=np.tile(gv[None, :], (P, 1)).astype(np.float32),
            )
        )

    return dict(
        NBLK=NBLK, NMAXP=NMAXP, HALF=HALF, gblk_u=gblk_u,
        mA=mA, mB=mB, eoffA=eoffA, eoffB=eoffB, EA=EA, EB=EB,
        CA=CA, CB=CB, ca0=ca0, ca1=ca1, cb0=cb0, cb1=cb1,
        nchA=nchA, nchB=nchB, nch=nch, dOff=dOff, DT=DT, NCHT=NCHT,
        gsz=gsz, cores=cores, deg=deg,
    )


def build(meta, GBLK=4, SINGLE_PACKET=False, ABLATE=(), REPS=1, QUEUES=4,
          GBUFS=3, QSPLIT=2):
    NBLK, NMAXP, HALF = meta["NBLK"], meta["NMAXP"], meta["HALF"]
    CA, CB, NCHT, DT = meta["CA"], meta["CB"], meta["NCHT"], meta["DT"]
    ca0 = [int(v) for v in meta["ca0"]]
    ca1 = [int(v) for v in meta["ca1"]]
    cb0 = [int(v) for v in meta["cb0"]]
    cb1 = [int(v) for v in meta["cb1"]]
    nchA = [int(v) for v in meta["nchA"]]
    nchB = [int(v) for v in meta["nchB"]]
    dOff = [int(v) for v in meta["dOff"]]
    gblk_u = meta["gblk_u"]
    NTAB = NCORE * NMAXP
    ngrp = (NBLK + GBLK - 1) // GBLK

    nc = bacc.Bacc(None, target_bir_lowering=False, num_swdge_queues=QUEUES)

    xT_d = nc.dram_tensor("xT", [2, NMAXP], BF16, kind="ExternalInput")
    degT_d = nc.dram_tensor("degT", [128, NBLK], BF16, kind="ExternalInput")
    degxT_d = nc.dram_tensor("degxT", [2, NMAXP], BF16, kind="ExternalInput")
    idx_d = nc.dram_tensor("idx", [128, NCHT * 8], I16, kind="ExternalInput")
    dstloc_d = nc.dram_tensor("dstloc", [128, DT], BF16, kind="ExternalInput")
    colidx_d = nc.dram_tensor("colidx", [128, 128], BF16, kind="ExternalInput")
    identbf_d = nc.dram_tensor("identbf", [128, 128], BF16, kind="ExternalInput")
    padmask_d = nc.dram_tensor("padmask", [128, NMAXP], BF16, kind="ExternalInput")
    gvalid_d = nc.dram_tensor("gvalid", [128, GPC], F32, kind="ExternalInput")
    gcnt_d = nc.dram_tensor("gcnt", [128, GPC], F32, kind="ExternalInput")
    W1_d = nc.dram_tensor("W1", [2, H], F32, kind="ExternalInput")
    W2_d = nc.dram_tensor("W2", [H, H], F32, kind="ExternalInput")
    W3_d = nc.dram_tensor("W3", [H, H], F32, kind="ExternalInput")
    Wo_d = nc.dram_tensor("Wo", [H, 2, OC], F32, kind="ExternalInput")
    bvec_d = nc.dram_tensor("bvec", [128, 3], F32, kind="ExternalInput")
    bo_d = nc.dram_tensor("bo", [GPC, OC], F32, kind="ExternalInput")
    out_d = nc.dram_tensor("out", [GPC, OC], F32, kind="ExternalOutput")

    shard_d = [nc.dram_tensor(f"shard{L}", [NMAXP, H], BF16) for L in range(3)]
    table_d = [
        nc.dram_tensor(f"table{L}", [NTAB, H], BF16, addr_space="Shared")
        for L in range(3)
    ]

    PB = 4
    # group gather chunk ranges
    gA0 = [ca0[g * GBLK] for g in range(ngrp)]
    gA1 = [ca1[min(g * GBLK + GBLK, NBLK) - 1] for g in range(ngrp)]
    gB0 = [cb0[g * GBLK] for g in range(ngrp)]
    gB1 = [cb1[min(g * GBLK + GBLK, NBLK) - 1] for g in range(ngrp)]
    gAmax = max(a1 - a0 for a0, a1 in zip(gA0, gA1))
    gBmax = max(b1 - b0 for b0, b1 in zip(gB0, gB1))
    nchmax = max(nchA[b] + nchB[b] for b in range(NBLK))

    with tile.TileContext(nc) as tc, ExitStack() as ctx:
        const = ctx.enter_context(tc.tile_pool(name="const", bufs=1))
        resid = ctx.enter_context(tc.tile_pool(name="resid", bufs=1))
        gap = ctx.enter_context(tc.tile_pool(name="gap", bufs=GBUFS))
        gbp = ctx.enter_context(tc.tile_pool(name="gbp", bufs=GBUFS))
        ohp = ctx.enter_context(tc.tile_pool(name="ohp", bufs=3))
        wk = ctx.enter_context(tc.tile_pool(name="wk", bufs=3))
        poolbig = ctx.enter_context(tc.tile_pool(name="poolbig", bufs=1))
        prepps = ctx.enter_context(tc.tile_pool(name="prepps", bufs=2, space="PSUM"))
        aggps = ctx.enter_context(tc.tile_pool(name="aggps", bufs=4, space="PSUM"))
        headps = ctx.enter_context(tc.tile_pool(name="headps", bufs=1, space="PSUM"))

        nc.gpsimd.load_library(library_config.mlp)

        def load_const(dram, shape, dt):
            t = const.tile(shape, dt, tag=dram.name)
            nc.sync.dma_start(t[:], dram[:])
            return t

        xT_t = load_const(xT_d, [2, NMAXP], BF16)
        degT_t = load_const(degT_d, [128, NBLK], BF16)
        degxT_t = load_const(degxT_d, [2, NMAXP], BF16)
        idx_t = load_const(idx_d, [128, NCHT * 8], I16)
        dstloc_t = load_const(dstloc_d, [128, DT], BF16)
        colidx_t = load_const(colidx_d, [128, 128], BF16)
        identbf_t = load_const(identbf_d, [128, 128], BF16)
        padmask_t = load_const(padmask_d, [128, NMAXP], BF16)
        gvalid_t = load_const(gvalid_d, [128, GPC], F32)
        gcnt_t = load_const(gcnt_d, [128, GPC], F32)
        W1_t = load_const(W1_d, [2, H], F32)
        W2_t = load_const(W2_d, [H, H], F32)
        W3_t = load_const(W3_d, [H, H], F32)
        Wo_t = load_const(Wo_d, [H, 2, OC], F32)
        bvec_t = load_const(bvec_d, [128, 3], F32)
        bo_t = load_const(bo_d, [GPC, OC], F32)

        rec1 = wk.tile([128, NBLK], F32, tag="rec1")
        nc.vector.reciprocal(rec1[:], degT_t[:])
        dinv_t = resid.tile([128, NBLK], F32, tag="dinv")
        nc.scalar.sqrt(dinv_t[:], rec1[:])

        rec2 = poolbig.tile([2, NMAXP], BF16, tag="big")
        with nc.allow_low_precision(reason="1/deg of small exact ints; 0.4% ok"):
            nc.vector.reciprocal(rec2[:], degxT_t[:])
        dinvxT = poolbig.tile([2, NMAXP], BF16, tag="big2")
        nc.scalar.sqrt(dinvxT[:], rec2[:])

        y0 = poolbig.tile([2, NMAXP], BF16, tag="big")
        nc.vector.tensor_tensor(y0[:], xT_t[:], dinvxT[:], OP.mult)

        dinv_rep = resid.tile([128, NMAXP], BF16, tag="dinv_rep")
        nc.gpsimd.partition_broadcast(dinv_rep[:], dinvxT[0:1, :])

        W1b = const.tile([2, H], BF16, tag="W1b")
        nc.vector.tensor_copy(W1b[:], W1_t[:])
        W2b = const.tile([128, H], BF16, tag="W2b")
        nc.vector.tensor_copy(W2b[:], W2_t[:])
        W3b = const.tile([128, H], BF16, tag="W3b")
        nc.vector.tensor_copy(W3b[:], W3_t[:])

        sbuild = resid.tile([128, NBLK, H], BF16, tag="sbuild")
        y_t = resid.tile([128, NMAXP], BF16, tag="y")

        for _rep in range(REPS):
         for L in range(3):
             Wb = (W1b, W2b, W3b)[L]
             for b0 in range(0, NBLK, PB):
                 b1 = min(b0 + PB, NBLK)
                 nb = b1 - b0
                 pp = prepps.tile([128, PB, H], F32, tag="pp")
                 for b in range(b0, b1):
                     lhs = y0[:, b * 128 : (b + 1) * 128] if L == 0 else \
                         y_t[:, b * 128 : (b + 1) * 128]
                     nc.tensor.matmul(pp[:, b - b0, :], lhs, Wb[:],
                                      start=True, stop=True)
                 if L == 0:
                     nc.vector.tensor_copy(sbuild[:, b0:b1, :], pp[:, :nb, :])
                 else:
                     nc.vector.tensor_tensor(
                         sbuild[:, b0:b1, :], pp[:, :nb, :],
                         dinv_t[:, b0:b1, None].broadcast_to((128, nb, H)),
                         OP.mult,
                     )
             nc.sync.dma_start(
                 shard_d[L].rearrange("(p b) h -> p (b h)", b=NBLK)[:, :],
                 sbuild[:].rearrange("p b h -> p (b h)"),
             )
             if "ag" not in ABLATE:
                 nc.gpsimd.collective_compute(
                     "AllGather", OP.bypass,
                     replica_groups=[list(range(NCORE))],
                     ins=[shard_d[L][:]], outs=[table_d[L][:]],
                 )

             for g in range(ngrp):
                 b0, b1 = g * GBLK, min((g + 1) * GBLK, NBLK)
                 nA = gA1[g] - gA0[g]
                 nB = gB1[g] - gB0[g]
                 need_g = "gather" not in ABLATE or "mm" not in ABLATE
                 gA = (gap.tile([128, gAmax, H], BF16, tag="gA", name="gA")
                       if nA and need_g else None)
                 gB = (gbp.tile([128, gBmax, H], BF16, tag="gB", name="gB")
                       if nB and need_g else None)
                 qg = 2 * g * QSPLIT
                 if nA and "gather" not in ABLATE:
                     for s in range(QSPLIT):
                         c0, c1 = (nA * s) // QSPLIT, (nA * (s + 1)) // QSPLIT
                         if c1 > c0:
                             nc.gpsimd.dma_gather(
                                 gA[:, c0:c1, :], table_d[L][0:HALF, :],
                                 idx_t[:, (gA0[g] + c0) * 8 : (gA0[g] + c1) * 8],
                                 (c1 - c0) * 128, (c1 - c0) * 128, H,
                                 single_packet=SINGLE_PACKET,
                                 queue_num=(qg + s) % QUEUES,
                             )
                 if nB and "gather" not in ABLATE:
                     for s in range(QSPLIT):
                         c0, c1 = (nB * s) // QSPLIT, (nB * (s + 1)) // QSPLIT
                         if c1 > c0:
                             nc.gpsimd.dma_gather(
                                 gB[:, c0:c1, :], table_d[L][HALF:, :],
                                 idx_t[:, (CA + gB0[g] + c0) * 8
                                       : (CA + gB0[g] + c1) * 8],
                                 (c1 - c0) * 128, (c1 - c0) * 128, H,
                                 single_packet=SINGLE_PACKET,
                                 queue_num=(qg + QSPLIT + s) % QUEUES,
                             )
                 for b in range(b0, b1):
                     na = nchA[b] if "mm" not in ABLATE else 0
                     nb_ = nchB[b] if "mm" not in ABLATE else 0
                     nch = na + nb_
                     if nch:
                         oh = ohp.tile([128, nchmax, 128], BF16, tag="oh",
                                       name="oh")
                         nc.vector.tensor_tensor(
                             oh[:, :nch, :],
                             colidx_t[:, None, :].broadcast_to((128, nch, 128)),
                             dstloc_t[:, dOff[b] : dOff[b] + nch, None]
                             .broadcast_to((128, nch, 128)),
                             OP.is_equal,
                         )
                     ap = aggps.tile([128, 128], F32, tag="agg")
                     for j in range(na):
                         nc.tensor.matmul(
                             ap[:], gA[:, ca0[b] - gA0[g] + j, :],
                             oh[:, j, :], start=(j == 0), stop=False,
                         )
                     for j in range(nb_):
                         nc.tensor.matmul(
                             ap[:], gB[:, cb0[b] - gB0[g] + j, :],
                             oh[:, na + j, :],
                             start=(na == 0 and j == 0), stop=False,
                         )
                     nc.tensor.matmul(
                         ap[:], sbuild[:, b, :], identbf_t[:],
                         start=(nch == 0), stop=True,
                     )
                     z = wk.tile([128, 128], F32, tag="z")
                     nc.vector.tensor_tensor(
                         z[:], ap[:], dinv_rep[:, b * 128 : (b + 1) * 128],
                         OP.mult,
                     )
                     nc.scalar.activation(
                         y_t[:, b * 128 : (b + 1) * 128], z[:], AF.Tanh,
                         bias=bvec_t[:, L : L + 1],
                     )

         gb = gblk_u * 128
         mx = resid.tile([128, GPC], F32, tag="mx")
         sm = resid.tile([128, GPC], F32, tag="sm")
         for g in range(GPC):
             zg = wk.tile([128, gb], F32, tag="zg")
             nc.vector.tensor_scalar(
                 zg[:], y_t[:, g * gb : (g + 1) * gb], 2.0, None, OP.add)
             nc.vector.tensor_tensor(
                 zg[:], zg[:], padmask_t[:, g * gb : (g + 1) * gb], OP.mult)
             nc.vector.tensor_reduce(
                 mx[:, g : g + 1], zg[:], mybir.AxisListType.X, OP.max)
             nc.vector.tensor_reduce(
                 sm[:, g : g + 1], zg[:], mybir.AxisListType.X, OP.add)
         recg = wk.tile([128, GPC], F32, tag="recg")
         nc.vector.reciprocal(recg[:], gcnt_t[:])
         mean2 = resid.tile([128, GPC], F32, tag="mean2")
         nc.vector.tensor_tensor(mean2[:], sm[:], recg[:], OP.mult)
         nc.vector.tensor_scalar(mean2[:], mean2[:], 2.0, None, OP.subtract)
         nc.vector.tensor_tensor(mean2[:], mean2[:], gvalid_t[:], OP.mult)
         mx2 = resid.tile([128, GPC], F32, tag="mx2")
         nc.vector.tensor_scalar(mx2[:], mx[:], 2.0, None, OP.subtract)
         nc.vector.tensor_tensor(mx2[:], mx2[:], gvalid_t[:], OP.mult)

         headp = headps.tile([GPC, OC], F32, tag="head")
         nc.tensor.matmul(headp[:], mx2[:], Wo_t[:, 0, :], start=True, stop=False)
         nc.tensor.matmul(headp[:], mean2[:], Wo_t[:, 1, :], start=False, stop=True)
         hsum = wk.tile([GPC, OC], F32, tag="hsum")
         nc.vector.tensor_tensor(hsum[:], headp[:], bo_t[:], OP.add)
         ofin = wk.tile([GPC, OC], F32, tag="ofin")
         nc.scalar.activation(ofin[:], hsum[:], AF.Tanh)
         nc.sync.dma_start(out_d[:], ofin[:])

    nc.compile()
    return nc


def make_in_maps(meta, inputs):
    colidx = np.tile(np.arange(128, dtype=np.float32), (128, 1)).astype(
        ml_dtypes.bfloat16)
    identbf = np.eye(128, dtype=np.float32).astype(ml_dtypes.bfloat16)
    bvec = np.stack(
        [np.asarray(inputs[b], np.float32) for b in ("b1", "b2", "b3")], 1)
    bo_t = np.tile(np.asarray(inputs["bo"], np.float32), (GPC, 1))
    Wo = np.asarray(inputs["Wo"], np.float32)
    Wo_t = np.ascontiguousarray(np.stack([Wo[:H], Wo[H:]], axis=1))
    gsz = meta["gsz"]
    maps = []
    for k, c in enumerate(meta["cores"]):
        gcnt = np.maximum(gsz[k * GPC : (k + 1) * GPC].astype(np.float32), 1.0)
        maps.append({
            "xT": np.asarray(c["xT"]),
            "degT": np.asarray(c["degT"]).astype(ml_dtypes.bfloat16),
            "degxT": np.asarray(c["degxT"]).astype(ml_dtypes.bfloat16),
            "idx": np.asarray(c["idx"]),
            "dstloc": np.asarray(c["dstloc"]),
            "colidx": colidx,
            "identbf": identbf,
            "padmask": np.asarray(c["padmask"]),
            "gvalid": np.asarray(c["gvalid"]),
            "gcnt": np.tile(gcnt[None, :], (128, 1)).astype(np.float32),
            "W1": np.asarray(inputs["W1"], np.float32),
            "W2": np.asarray(inputs["W2"], np.float32),
            "W3": np.asarray(inputs["W3"], np.float32),
            "Wo": Wo_t,
            "bvec": bvec.astype(np.float32),
            "bo": bo_t,
        })
    return maps


_CACHE = {}


def kernel(x, edge_index, batch, W1, b1, W2, b2, W3, b3, Wo, bo):
    x = np.asarray(x, np.float32)
    meta = prep(x, np.asarray(edge_index), np.asarray(batch), 64)
    key = (meta["NBLK"], meta["CA"], meta["CB"], tuple(meta["dOff"]))
    if key not in _CACHE:
        _CACHE[key] = build(meta)
    nc = _CACHE[key]
    inputs = dict(W1=W1, b1=b1, W2=W2, b2=b2, W3=W3, b3=b3, Wo=Wo, bo=bo)
    in_maps = make_in_maps(meta, inputs)
    res = run_bass_kernel_spmd(nc, in_maps, core_ids=list(range(8)), trace=False)
    out = np.concatenate([res.results[k]["out"] for k in range(8)], 0)
    return np.ascontiguousarray(out, dtype=np.float32)
